# revision 8
# baseline (speedup 1.0000x reference)
"""CrossAttention TRN2 kernel.

Full-input contract: kernel(**inputs) takes the unsharded numpy inputs of
  reference.py (q,k,v [2,2048,1024] fp32; Wq/Wk/Wv/Wo [1024,1024]; biases)
and returns the full [2,2048,1024] fp32 output.

Sharding: 8 cores = 2 batch groups x 4 head groups (tensor parallel over
heads).  Core c handles batch c//4 and heads [4*(c%4), 4*(c%4)+4).
Each core computes its heads' Q/K/V projections, attention, and a partial
output projection (row-slice of Wo); the host sums the 4 partials per batch
(no on-device collectives needed).

Per-core dataflow (all matmuls bf16 with fp32 PSUM accumulation):
  - host pre-transposes/casts activations (q^T,k^T,v^T [cin, tok] bf16) and
    weight slices, so contraction dims land on SBUF partitions directly.
  - scores are computed transposed ([ts, tq]) so the PV matmul can contract
    ts on partitions; a ones-column appended to vh yields the softmax
    denominator as PV row 64 for free.
  - exp runs on ScalarE (scale 1/sqrt(d) folded in), FD=1024 per activation.

Schedule (ScalarE exp is the pacing engine at ~1.34us per [128,1024] tile):
  attention pairs start right after the minimal projection prefix
  (K chunk 0 + Q chunks 0..1); all remaining projection chunks, the 16
  V-proj tiles and the tb0 out-projection are spread as per-slot fillers
  inside the pair iterations so the PE fills ScalarE-gated stalls without
  ever running far ahead.  The last pair self-interleaves its own PV for
  BOTH 512-column chunks so the epilogue is only normalize + out-proj.
"""

import os
import numpy as np
import ml_dtypes

BF16 = ml_dtypes.bfloat16

B, TOKENS, C = 2, 2048, 1024
NHEAD, D = 16, 64
NCORES = 8
NGROUP = 4                # head groups (cores per batch)
COUT = C // NGROUP        # 256 head-channels per core
NH = NHEAD // NGROUP      # 4 heads per core

P = 128                   # SBUF partitions


def build_nc(tok=TOKENS, cin=C, cout=COUT, nh=NH):
    """Emit the per-core Bass module. Parametric so a small version can be
    validated in CoreSim quickly. d=64 fixed; cout = nh*64."""
    import concourse.bacc as bacc
    import concourse.tile as tile
    import concourse.mybir as mybir

    d = D
    assert cout == nh * d
    ncin = cin // P               # cin tiles (contraction)
    nt = tok // P                 # token tiles
    nm = max(1, cout // P)        # 128-wide cout chunks (qhT/khT)
    tqb = min(1024, tok)          # tq block (exp FD)
    ntqb = tok // tqb
    sck = min(512, tok)           # matmul moving chunk
    csk = tqb // sck              # chunks per tq block
    nchunk = tok // sck           # qk chunks per m
    nob = max(1, min(2, cin // 512))  # out-proj cout chunks of 512
    ob = cin // nob               # out-proj N per chunk
    nko = cout // P if cout >= P else 1  # out-proj contraction tiles

    fp32 = mybir.dt.float32
    bf16 = mybir.dt.bfloat16

    nc = bacc.Bacc("TRN2", target_bir_lowering=False, debug=False)

    qT = nc.dram_tensor("qT", [cin, tok], bf16, kind="ExternalInput")
    kT = nc.dram_tensor("kT", [cin, tok], bf16, kind="ExternalInput")
    vT = nc.dram_tensor("vT", [cin, tok], bf16, kind="ExternalInput")
    wqT = nc.dram_tensor("wqT", [cin, cout], bf16, kind="ExternalInput")
    wkT = nc.dram_tensor("wkT", [cin, cout], bf16, kind="ExternalInput")
    wvT = nc.dram_tensor("wvT", [cin, cout], bf16, kind="ExternalInput")
    woT = nc.dram_tensor("woT", [cout, cin], bf16, kind="ExternalInput")
    bqv = nc.dram_tensor("bqv", [P, nm], fp32, kind="ExternalInput")
    bkv = nc.dram_tensor("bkv", [P, nm], fp32, kind="ExternalInput")
    bvv = nc.dram_tensor("bvv", [1, cout], fp32, kind="ExternalInput")
    outp = nc.dram_tensor("outp", [tok, cin], bf16, kind="ExternalOutput")

    with tile.TileContext(nc) as tc:
        from contextlib import ExitStack
        with ExitStack() as ctx:
            consts = ctx.enter_context(tc.tile_pool(name="consts", bufs=1))
            xstream = ctx.enter_context(tc.tile_pool(name="xstream", bufs=2))
            vstream = ctx.enter_context(tc.tile_pool(name="vstream", bufs=2))
            persist = ctx.enter_context(tc.tile_pool(name="persist", bufs=1))
            expool = ctx.enter_context(tc.tile_pool(name="expool", bufs=4))
            smalls = ctx.enter_context(tc.tile_pool(name="smalls", bufs=4))
            ostage = ctx.enter_context(tc.tile_pool(name="ostage", bufs=4))
            psum = ctx.enter_context(
                tc.tile_pool(name="psum", bufs=1, space="PSUM"))

            # ---- constants (K/Q weights first — they gate the first exp) ---
            wq_sb = consts.tile([P, ncin, cout], bf16, tag="wq")
            wk_sb = consts.tile([P, ncin, cout], bf16, tag="wk")
            wv_sb = consts.tile([P, ncin, cout], bf16, tag="wv")
            weng = nc.sync if os.environ.get("K_SYNC_CONSTS") else nc.scalar
            wengl = nc.sync if os.environ.get("K_SYNC_CONSTS") else nc.gpsimd
            for w_sb, w_h in ((wk_sb, wkT), (wq_sb, wqT)):
                weng.dma_start(
                    out=w_sb,
                    in_=w_h[:, :].rearrange("(nb p) co -> p nb co", p=P))
            bq_sb = consts.tile([P, nm], fp32, tag="bq")
            bk_sb = consts.tile([P, nm], fp32, tag="bk")
            nc.sync.dma_start(out=bq_sb, in_=bqv[:, :])
            nc.sync.dma_start(out=bk_sb, in_=bkv[:, :])
            # V/O weights + bv load behind the first Q/K activation chunks
            # (emitted below, scheduled after by SP queue order).
            wo_sb = consts.tile([P, nko, cin], bf16, tag="wo")
            bv_sb = consts.tile([P, nh, d], fp32, tag="bv")

            def emit_late_consts():
                wengl.dma_start(
                    out=wv_sb,
                    in_=wvT[:, :].rearrange("(nb p) co -> p nb co", p=P))
                wengl.dma_start(
                    out=wo_sb,
                    in_=woT[:, :].rearrange("(nb p) co -> p nb co", p=P))
                nc.gpsimd.dma_start(
                    out=bv_sb,
                    in_=bvv[:, :].rearrange("o (h e) -> o h e", h=nh)
                    .to_broadcast([P, nh, d]))

            # ---- projections ----------------------------------------------
            vh_all = persist.tile([P, nt, nh, d + 1], bf16, tag="vh")
            nc.vector.memset(vh_all[:, :, :, d:d + 1], 1.0)
            qh_sb = persist.tile([P, nm, tok], bf16, tag="qh")
            kh_sb = persist.tile([P, nm, tok], bf16, tag="kh")

            # Streaming input DMAs round-robin over the SP and GpSimd
            # queues — a single queue moves ~100 GB/s, so the 1MB
            # activation chunks serialize into a 26us startup stall
            # otherwise.  (Only SP/Activation/GpSimd can initiate DMAs;
            # the Activation queue belongs to the pacing ScalarE and is
            # only borrowed for the prologue, before the exp stream.)
            dmaq = [nc.sync, nc.gpsimd]
            dmaq_i = [0]
            prologue_q = [nc.sync, nc.gpsimd, nc.scalar]

            def next_q():
                if prologue_q:
                    return prologue_q.pop(0)
                q = dmaq[dmaq_i[0] % len(dmaq)]
                dmaq_i[0] += 1
                return q

            def emit_qk_chunk(x_h, w_sb, b_sb, xh_sb, it, m, xtag):
                xt = xstream.tile([P, ncin, sck], bf16, tag=xtag,
                                  name=f"xt_{xtag}_{it}_{m}")
                next_q().dma_start(
                    out=xt,
                    in_=x_h[:, :].rearrange("(nb p) t -> p nb t", p=P)
                    [:, :, it * sck:(it + 1) * sck])
                ps = psum.tile([P, sck], fp32, tag="pp", bufs=2, name="psqk")
                for ci in range(ncin):
                    nc.tensor.matmul(
                        ps, w_sb[:, ci, m * P:(m + 1) * P], xt[:, ci, :],
                        start=(ci == 0), stop=(ci == ncin - 1))
                nc.vector.tensor_scalar(
                    out=xh_sb[:, m, it * sck:(it + 1) * sck],
                    in0=ps, scalar1=b_sb[:, m:m + 1], scalar2=None,
                    op0=mybir.AluOpType.add)

            def emit_v_tile(it):
                vt = vstream.tile([P, ncin, P], bf16, tag="vt",
                                  name=f"vt_{it}")
                next_q().dma_start(
                    out=vt,
                    in_=vT[:, :].rearrange("(nb p) t -> p nb t", p=P)
                    [:, :, it * P:(it + 1) * P])
                ps = psum.tile([P, cout], fp32, tag="pp", bufs=2, name="psv")
                for ci in range(ncin):
                    nc.tensor.matmul(ps, vt[:, ci, :], wv_sb[:, ci, :],
                                     start=(ci == 0), stop=(ci == ncin - 1))
                nc.vector.tensor_tensor(
                    out=vh_all[:, it, :, 0:d],
                    in0=ps.rearrange("p (h e) -> p h e", h=nh),
                    in1=bv_sb,
                    op=mybir.AluOpType.add)

            # Minimal projection prefix: only what the first scores matmul
            # of pair (tb0, hp0) needs — kh m0 tokens 0:sck and qh m0 tokens
            # 0:tqb.  Everything else is a filler inside the pair slots.
            emit_qk_chunk(kT, wk_sb, bk_sb, kh_sb, 0, 0, "xk")
            for it in range(tqb // sck):
                emit_qk_chunk(qT, wq_sb, bq_sb, qh_sb, it, 0, "xq")
            emit_late_consts()

            def mk_qk(x, it, m):
                if x == 'k':
                    return lambda: emit_qk_chunk(kT, wk_sb, bk_sb, kh_sb,
                                                 it, m, "xk")
                return lambda: emit_qk_chunk(qT, wq_sb, bq_sb, qh_sb,
                                             it, m, "xq")

            def mk_v(it):
                return lambda: emit_v_tile(it)

            # Per-(pair, slot) filler schedule.  Constraints: kh chunk
            # (it, m) before pair-of-m slot 4*it; qh chunks of a tq block
            # before the pair that consumes them; all V tiles before the
            # first PV sweep reaches them (pair1 slot 7 uses vh[14:16]).
            fill = {}

            def addf(p, s, th):
                fill.setdefault((p, s), []).append(th)

            full_sched = (nt == 16 and nm == 2 and nchunk == 4 and ntqb == 2
                          and nh == 4)
            if full_sched:
                addf(0, 1, mk_qk('k', 1, 0))
                addf(0, 3, mk_qk('k', 2, 0))
                addf(0, 5, mk_qk('k', 3, 0))
                addf(0, 7, mk_qk('q', 2, 0))
                addf(0, 9, mk_qk('k', 0, 1))
                addf(0, 11, mk_qk('q', 0, 1))
                addf(0, 13, mk_qk('q', 1, 1))
                for j, s in enumerate((2, 4, 6, 8, 10, 12, 14, 15)):
                    addf(0, s, mk_v(j))                    # V0..V7
                addf(1, 0, mk_qk('k', 1, 1))
                for j, s in enumerate((0, 1, 2, 3, 4, 5, 6, 6)):
                    addf(1, s, mk_v(8 + j))                # V8..V15
                addf(1, 7, mk_qk('k', 2, 1))
                addf(1, 9, mk_qk('q', 3, 0))
                addf(1, 11, mk_qk('k', 3, 1))
                addf(2, 1, mk_qk('q', 2, 1))
                addf(2, 3, mk_qk('q', 3, 1))
            else:
                # correctness fallback (small CoreSim configs): emit all
                # remaining chunks upfront; V tiles stream in pair 0.
                for m in range(nm):
                    for it in range(nchunk):
                        if m == 0 and it == 0:
                            continue
                        if m == 0 and it < tqb // sck:
                            mk_qk('k', it, 0)()
                            continue
                        mk_qk('k', it, m)()
                        mk_qk('q', it, m)()

            # ---- attention per head ---------------------------------------
            att_pair = [persist.tile([P, tok], bf16, tag=f"att{k}",
                                     name=f"att{k}")
                        for k in range(nko)]
            # Attention runs in head-pairs (even head on partitions 0:64,
            # odd on 64:128 — adjacent matmuls can row-tile concurrently on
            # the PE).  Phase 1 streams scores->exp into SBUF for the whole
            # pair (ScalarE stays saturated, nothing gates on PV); phase 2
            # does the PV accumulations at [65, sck] (one PSUM bank each)
            # and is interleaved, slot by slot, into the NEXT pair's phase 1
            # so it fills PE slack instead of stalling the exp stream.
            exp_bufs = 2 * nt + 6

            def emit_normalize(tb, m, h, p0, ck, stg):
                # reciprocal + GpSimd partition-broadcast + multiply.
                # NB: the custom-DVE reciprocal gets a partition-0 operand —
                # feeding it stg[64:65] directly breaks on hardware (passes
                # CoreSim), so copy the denominator row down first.
                den = smalls.tile([1, sck], fp32, tag="den",
                                  name=f"den_{tb}_{h}_{ck}")
                nc.vector.tensor_copy(out=den, in_=stg[d:d + 1, :])
                rec = smalls.tile([1, sck], fp32, tag="rec",
                                  name=f"rec_{tb}_{h}_{ck}")
                nc.vector.reciprocal_approx_fast(out=rec, in_=den)
                rep = smalls.tile([d, sck], fp32, tag="rep",
                                  name=f"rep_{tb}_{h}_{ck}")
                nc.gpsimd.partition_broadcast(out_ap=rep, in_ap=rec)
                c0 = tb * tqb + ck * sck
                nc.vector.tensor_tensor(
                    out=att_pair[m][p0:p0 + d, c0:c0 + sck],
                    in0=stg[0:d, :], in1=rep,
                    op=mybir.AluOpType.mult)

            def make_phase2_slots(tb, m, heads, exs):
                # Distribute the pair's PV work over nt emission slots,
                # ck-major when csk==2: first half of slots advances BOTH
                # heads' ck0 accumulators (2 ts tiles each), second half
                # ck1.  Each ck's normalize lands at mid-pair / pair-end,
                # so the downstream out-projection of that 512-token chunk
                # can start half a pair early instead of bursting at the
                # end.  Falls back to head-major for csk != 2.
                half = nt // 2
                state = {}

                def slot_ck(s):
                    ck = 0 if s < half else 1
                    if (s % half) == 0:
                        for hi, _ in enumerate(heads):
                            state[hi] = psum.tile(
                                [d + 1, sck], fp32, tag="pv", bufs=2,
                                name=f"pv_{tb}_{hi}_{ck}")
                    base = (s % half) * 2
                    for hi, (h, p0) in enumerate(heads):
                        for ts in (base, base + 1):
                            nc.tensor.matmul(
                                state[hi], vh_all[:, ts, h, :],
                                exs[(h, ts)][:, ck * sck:(ck + 1) * sck],
                                start=(ts == 0), stop=(ts == nt - 1))
                    if (s % half) == half - 1:
                        for hi, (h, p0) in enumerate(heads):
                            stg = smalls.tile([d + 1, sck], fp32, tag="stg",
                                              name=f"stg_{tb}_{h}_{ck}")
                            nc.vector.tensor_copy(out=stg, in_=state[hi])
                            emit_normalize(tb, m, h, p0, ck, stg)

                def slot_h(s):
                    h, p0 = heads[0] if s < half else heads[1]
                    if (s % half) == 0:
                        state[h] = [psum.tile([d + 1, sck], fp32, tag="pv",
                                              bufs=2,
                                              name=f"pv_{tb}_{h}_{ck}")
                                    for ck in range(csk)]
                    base = (s % half) * 2
                    for ck in range(csk):
                        for ts in (base, base + 1):
                            nc.tensor.matmul(
                                state[h][ck], vh_all[:, ts, h, :],
                                exs[(h, ts)][:, ck * sck:(ck + 1) * sck],
                                start=(ts == 0), stop=(ts == nt - 1))
                    if (s % half) == half - 1:
                        for ck in range(csk):
                            stg = smalls.tile([d + 1, sck], fp32, tag="stg",
                                              name=f"stg_{tb}_{h}_{ck}")
                            nc.vector.tensor_copy(out=stg, in_=state[h][ck])
                            emit_normalize(tb, m, h, p0, ck, stg)
                return slot_ck if (csk == 2 and nh == 4) else slot_h

            def emit_outproj_unit(tt, n, copy_eng):
                ps = psum.tile([P, ob], fp32, tag="pp", bufs=2, name="pso")
                for ko in range(nko):
                    nc.tensor.matmul(
                        ps, att_pair[ko][:, tt * P:(tt + 1) * P],
                        wo_sb[:, ko, n * ob:(n + 1) * ob],
                        start=(ko == 0), stop=(ko == nko - 1))
                o_sb = ostage.tile([P, ob], bf16, tag="ost")
                copy_eng(out=o_sb, in_=ps)
                nc.sync.dma_start(
                    out=outp[tt * P:(tt + 1) * P, n * ob:(n + 1) * ob],
                    in_=o_sb)

            def emit_outproj(tb, alternate=False):
                u = 0
                for tt in range(tb * (tqb // P), (tb + 1) * (tqb // P)):
                    for n in range(nob):
                        if alternate and u % 2 == 1:
                            emit_outproj_unit(tt, n, nc.scalar.copy)
                        else:
                            emit_outproj_unit(
                                tt, n,
                                lambda out, in_: nc.vector.tensor_copy(
                                    out=out, in_=in_))
                        u += 1

            pairs = [(tb, hp) for tb in range(ntqb) for hp in range(nh // 2)]
            last_idx = len(pairs) - 1
            pending = None        # (slot_fn, tb, was_last_in_tb, exs)
            self_pv = None        # [ck][hi] PSUM accumulators for last pair
            for idx, (tb, hp) in enumerate(pairs):
                m = hp if nm > 1 else 0
                heads = ((2 * hp, 0), (2 * hp + 1, d))
                is_last = (idx == last_idx and nt >= 16 and csk == 2
                           and not os.environ.get("K_NO_SELFPV"))
                exs = {}
                for i in range(nt):
                    for h, p0 in heads:
                        s_ps = psum.tile([P, tqb], fp32, tag="s",
                                         bufs=2, name="s_ps")
                        for cc in range(csk):
                            q0 = tb * tqb + cc * sck
                            nc.tensor.matmul(
                                s_ps[:, cc * sck:(cc + 1) * sck],
                                kh_sb[p0:p0 + d, m, i * P:(i + 1) * P],
                                qh_sb[p0:p0 + d, m, q0:q0 + sck],
                                start=True, stop=True)
                        ex = expool.tile([P, tqb], bf16, tag="ex",
                                         bufs=exp_bufs, name=f"ex_{h}_{i}")
                        nc.scalar.activation(
                            out=ex, in_=s_ps,
                            func=mybir.ActivationFunctionType.Exp,
                            scale=float(d) ** -0.5)
                        exs[(h, i)] = ex
                    for th in fill.get((idx, i), []):
                        th()
                    if not full_sched and idx == 0 and i < nt:
                        emit_v_tile(i)
                    if pending is not None and not os.environ.get(
                            "K_NO_INTERLEAVE"):
                        if is_last:
                            # compress the previous pair's drain into the
                            # first half so the final pair's own PV can
                            # self-interleave into the second half.
                            if i < nt // 2:
                                pending[0](2 * i)
                                pending[0](2 * i + 1)
                        else:
                            pending[0](i)
                    if (pending is not None and pending[2] and csk == 2
                            and not is_last and i >= nt // 2
                            and not os.environ.get("K_NO_INTERLEAVE")):
                        # ck0 of the previous pair's tb normalized at mid
                        # pair — spread that half of its out-projection
                        # here, one unit per slot (VectorE copies only;
                        # ScalarE is pacing the exp stream).
                        u = i - nt // 2
                        tt = pending[1] * (tqb // P) + u // nob
                        emit_outproj_unit(
                            tt, u % nob,
                            lambda out, in_: nc.vector.tensor_copy(
                                out=out, in_=in_))
                    if is_last and i >= nt // 2:
                        if i == nt // 2:
                            # ck0 accumulators reuse the freed "pv" bufs,
                            # ck1 the freed "pp" bufs (outproj burst done).
                            self_pv = [
                                [psum.tile([d + 1, sck], fp32,
                                           tag=("pv" if ck == 0 else "pp"),
                                           bufs=2, name=f"pvsi_{ck}_{h2}")
                                 for h2, _ in heads]
                                for ck in range(csk)]
                        for hi, (h2, _) in enumerate(heads):
                            for ts in (2 * (i - nt // 2),
                                       2 * (i - nt // 2) + 1):
                                for ck in range(csk):
                                    nc.tensor.matmul(
                                        self_pv[ck][hi],
                                        vh_all[:, ts, h2, :],
                                        exs[(h2, ts)][:,
                                                      ck * sck:(ck + 1) * sck],
                                        start=(ts == 0), stop=(ts == nt - 1))
                if pending is not None and pending[2]:
                    if csk == 2 and not os.environ.get("K_NO_INTERLEAVE"):
                        # ck0 units were spread into the slots above; emit
                        # only the ck1 half (normalized at pair end) here.
                        base = pending[1] * (tqb // P) + sck // P
                        u = 0
                        for tt in range(base, base + sck // P):
                            for n in range(nob):
                                if u % 2 == 1:
                                    emit_outproj_unit(tt, n, nc.scalar.copy)
                                else:
                                    emit_outproj_unit(
                                        tt, n,
                                        lambda out, in_:
                                        nc.vector.tensor_copy(
                                            out=out, in_=in_))
                                u += 1
                    else:
                        emit_outproj(pending[1], alternate=True)
                pending = (make_phase2_slots(tb, m, heads, exs), tb,
                           hp == nh // 2 - 1, exs)
                if os.environ.get("K_NO_INTERLEAVE") and idx != last_idx:
                    for s_i in range(nt):
                        pending[0](s_i)
            # Drain: all of the last pair's PV is already accumulated in
            # self_pv (both chunks), so the epilogue is normalize for every
            # (head, ck) first — releasing all self_pv PSUM (incl. the
            # pp-tagged ck1 tiles the out-proj needs) — then the final
            # out-projection with PSUM->SBUF copies alternating between
            # ScalarE (idle after the last exp) and VectorE.
            tb_l = pending[1]
            hp_l = nh // 2 - 1
            m_l = hp_l if nm > 1 else 0
            heads_l = ((2 * hp_l, 0), (2 * hp_l + 1, d))
            exs_l = pending[3]
            for ck in range(csk):
                for hi, (h, p0) in enumerate(heads_l):
                    if self_pv is not None:
                        pv = self_pv[ck][hi]
                    else:
                        pv = psum.tile([d + 1, sck], fp32, tag="pv", bufs=2,
                                       name=f"pvf_{h}_{ck}")
                        for ts in range(nt):
                            nc.tensor.matmul(
                                pv, vh_all[:, ts, h, :],
                                exs_l[(h, ts)][:, ck * sck:(ck + 1) * sck],
                                start=(ts == 0), stop=(ts == nt - 1))
                    stg = smalls.tile([d + 1, sck], fp32, tag="stg",
                                      name=f"stgf_{h}_{ck}")
                    nc.vector.tensor_copy(out=stg, in_=pv)
                    emit_normalize(tb_l, m_l, h, p0, ck, stg)
            u = 0
            for ck in range(csk):
                c0 = (tb_l * tqb + ck * sck) // P
                for tt in range(c0, c0 + sck // P):
                    for n in range(nob):
                        if u % 2 == 0:
                            emit_outproj_unit(tt, n, nc.scalar.copy)
                        else:
                            emit_outproj_unit(
                                tt, n,
                                lambda out, in_: nc.vector.tensor_copy(
                                    out=out, in_=in_))
                        u += 1

    nc.compile()
    return nc


def _host_inputs(q, k, v, Wq, Wk, Wv, Wo, bq, bk, bv,
                 tok=TOKENS, cin=C, cout=COUT, ngroup=NGROUP, ncores=NCORES):
    """Build per-core in_maps (host-side shard + transpose + bf16 cast)."""
    nm = max(1, cout // P)
    xT = {}
    for b in range(q.shape[0]):
        xT[('q', b)] = np.ascontiguousarray(q[b].T).astype(BF16)
        xT[('k', b)] = np.ascontiguousarray(k[b].T).astype(BF16)
        xT[('v', b)] = np.ascontiguousarray(v[b].T).astype(BF16)
    in_maps = []
    for core in range(ncores):
        b, g = core // ngroup, core % ngroup
        sl = slice(g * cout, (g + 1) * cout)
        in_maps.append({
            "qT": xT[('q', b)],
            "kT": xT[('k', b)],
            "vT": xT[('v', b)],
            "wqT": np.ascontiguousarray(Wq[sl, :].T).astype(BF16),
            "wkT": np.ascontiguousarray(Wk[sl, :].T).astype(BF16),
            "wvT": np.ascontiguousarray(Wv[sl, :].T).astype(BF16),
            "woT": np.ascontiguousarray(Wo[:, sl].T).astype(BF16),
            "bqv": np.ascontiguousarray(
                bq[sl].reshape(nm, P).T).astype(np.float32),
            "bkv": np.ascontiguousarray(
                bk[sl].reshape(nm, P).T).astype(np.float32),
            "bvv": np.ascontiguousarray(bv[sl][None, :]).astype(np.float32),
        })
    return in_maps


_NC_CACHE = {}


def _get_nc():
    if "nc" not in _NC_CACHE:
        _NC_CACHE["nc"] = build_nc()
    return _NC_CACHE["nc"]


def kernel(q, k, v, Wq, bq, Wk, bk, Wv, bv, Wo, bo):
    from concourse.bass_utils import run_bass_kernel_spmd

    q = np.asarray(q, dtype=np.float32)
    k = np.asarray(k, dtype=np.float32)
    v = np.asarray(v, dtype=np.float32)
    nc = _get_nc()
    in_maps = _host_inputs(q, k, v,
                           np.asarray(Wq, np.float32), np.asarray(Wk, np.float32),
                           np.asarray(Wv, np.float32), np.asarray(Wo, np.float32),
                           np.asarray(bq, np.float32), np.asarray(bk, np.float32),
                           np.asarray(bv, np.float32))
    res = run_bass_kernel_spmd(nc, in_maps, core_ids=list(range(NCORES)))
    parts = [np.asarray(r["outp"], dtype=np.float32) for r in res.results]
    out = np.stack(
        [sum(parts[b * NGROUP:(b + 1) * NGROUP]) for b in range(B)], axis=0)
    out = out + np.asarray(bo, np.float32)[None, None, :]
    return out.astype(np.float32)


# revision 9
# speedup vs baseline: 1.0087x; 1.0087x over previous
"""CrossAttention TRN2 kernel.

Full-input contract: kernel(**inputs) takes the unsharded numpy inputs of
  reference.py (q,k,v [2,2048,1024] fp32; Wq/Wk/Wv/Wo [1024,1024]; biases)
and returns the full [2,2048,1024] fp32 output.

Sharding: 8 cores = 2 batch groups x 4 head groups (tensor parallel over
heads).  Core c handles batch c//4 and heads [4*(c%4), 4*(c%4)+4).
Each core computes its heads' Q/K/V projections, attention, and a partial
output projection (row-slice of Wo); the host sums the 4 partials per batch
(no on-device collectives needed).

Per-core dataflow (all matmuls bf16 with fp32 PSUM accumulation):
  - host pre-transposes/casts activations (q^T,k^T,v^T [cin, tok] bf16) and
    weight slices, so contraction dims land on SBUF partitions directly.
  - scores are computed transposed ([ts, tq]) so the PV matmul can contract
    ts on partitions; a ones-column appended to vh yields the softmax
    denominator as PV row 64 for free.
  - exp runs on ScalarE (scale 1/sqrt(d) folded in), FD=1024 per activation.

Schedule (ScalarE exp is the pacing engine at ~1.34us per [128,1024] tile):
  attention pairs start right after the minimal projection prefix
  (K chunk 0 + Q chunks 0..1); all remaining projection chunks, the 16
  V-proj tiles and the tb0 out-projection are spread as per-slot fillers
  inside the pair iterations so the PE fills ScalarE-gated stalls without
  ever running far ahead.  The last pair self-interleaves its own PV for
  BOTH 512-column chunks so the epilogue is only normalize + out-proj.
"""

import os
import numpy as np
import ml_dtypes

BF16 = ml_dtypes.bfloat16

B, TOKENS, C = 2, 2048, 1024
NHEAD, D = 16, 64
NCORES = 8
NGROUP = 4                # head groups (cores per batch)
COUT = C // NGROUP        # 256 head-channels per core
NH = NHEAD // NGROUP      # 4 heads per core

P = 128                   # SBUF partitions


def build_nc(tok=TOKENS, cin=C, cout=COUT, nh=NH):
    """Emit the per-core Bass module. Parametric so a small version can be
    validated in CoreSim quickly. d=64 fixed; cout = nh*64."""
    import concourse.bacc as bacc
    import concourse.tile as tile
    import concourse.mybir as mybir

    d = D
    assert cout == nh * d
    ncin = cin // P               # cin tiles (contraction)
    nt = tok // P                 # token tiles
    nm = max(1, cout // P)        # 128-wide cout chunks (qhT/khT)
    tqb = min(1024, tok)          # tq block (exp FD)
    ntqb = tok // tqb
    sck = min(512, tok)           # matmul moving chunk
    csk = tqb // sck              # chunks per tq block
    nchunk = tok // sck           # qk chunks per m
    nob = max(1, min(2, cin // 512))  # out-proj cout chunks of 512
    ob = cin // nob               # out-proj N per chunk
    nko = cout // P if cout >= P else 1  # out-proj contraction tiles

    fp32 = mybir.dt.float32
    bf16 = mybir.dt.bfloat16

    nc = bacc.Bacc("TRN2", target_bir_lowering=False, debug=False)

    qT = nc.dram_tensor("qT", [cin, tok], bf16, kind="ExternalInput")
    kT = nc.dram_tensor("kT", [cin, tok], bf16, kind="ExternalInput")
    vT = nc.dram_tensor("vT", [cin, tok], bf16, kind="ExternalInput")
    wqT = nc.dram_tensor("wqT", [cin, cout], bf16, kind="ExternalInput")
    wkT = nc.dram_tensor("wkT", [cin, cout], bf16, kind="ExternalInput")
    wvT = nc.dram_tensor("wvT", [cin, cout], bf16, kind="ExternalInput")
    woT = nc.dram_tensor("woT", [cout, cin], bf16, kind="ExternalInput")
    bqv = nc.dram_tensor("bqv", [P, nm], fp32, kind="ExternalInput")
    bkv = nc.dram_tensor("bkv", [P, nm], fp32, kind="ExternalInput")
    bvv = nc.dram_tensor("bvv", [1, cout], fp32, kind="ExternalInput")
    outp = nc.dram_tensor("outp", [tok, cin], bf16, kind="ExternalOutput")

    with tile.TileContext(nc) as tc:
        from contextlib import ExitStack
        with ExitStack() as ctx:
            consts = ctx.enter_context(tc.tile_pool(name="consts", bufs=1))
            xstream = ctx.enter_context(tc.tile_pool(name="xstream", bufs=2))
            vstream = ctx.enter_context(tc.tile_pool(name="vstream", bufs=2))
            persist = ctx.enter_context(tc.tile_pool(name="persist", bufs=1))
            expool = ctx.enter_context(tc.tile_pool(name="expool", bufs=4))
            smalls = ctx.enter_context(tc.tile_pool(name="smalls", bufs=4))
            ostage = ctx.enter_context(tc.tile_pool(name="ostage", bufs=4))
            psum = ctx.enter_context(
                tc.tile_pool(name="psum", bufs=1, space="PSUM"))

            # ---- constants (K/Q weights first — they gate the first exp) ---
            wq_sb = consts.tile([P, ncin, cout], bf16, tag="wq")
            wk_sb = consts.tile([P, ncin, cout], bf16, tag="wk")
            wv_sb = consts.tile([P, ncin, cout], bf16, tag="wv")
            weng = nc.sync if os.environ.get("K_SYNC_CONSTS") else nc.scalar
            wengl = nc.sync if os.environ.get("K_SYNC_CONSTS") else nc.gpsimd
            for w_sb, w_h in ((wk_sb, wkT), (wq_sb, wqT)):
                weng.dma_start(
                    out=w_sb,
                    in_=w_h[:, :].rearrange("(nb p) co -> p nb co", p=P))
            bq_sb = consts.tile([P, nm], fp32, tag="bq")
            bk_sb = consts.tile([P, nm], fp32, tag="bk")
            nc.sync.dma_start(out=bq_sb, in_=bqv[:, :])
            nc.sync.dma_start(out=bk_sb, in_=bkv[:, :])
            # V/O weights + bv load behind the first Q/K activation chunks
            # (emitted below, scheduled after by SP queue order).
            wo_sb = consts.tile([P, nko, cin], bf16, tag="wo")
            bv_sb = consts.tile([P, nh, d], fp32, tag="bv")

            def emit_late_consts():
                wengl.dma_start(
                    out=wv_sb,
                    in_=wvT[:, :].rearrange("(nb p) co -> p nb co", p=P))
                wengl.dma_start(
                    out=wo_sb,
                    in_=woT[:, :].rearrange("(nb p) co -> p nb co", p=P))
                nc.gpsimd.dma_start(
                    out=bv_sb,
                    in_=bvv[:, :].rearrange("o (h e) -> o h e", h=nh)
                    .to_broadcast([P, nh, d]))

            # ---- projections ----------------------------------------------
            vh_all = persist.tile([P, nt, nh, d + 1], bf16, tag="vh")
            nc.vector.memset(vh_all[:, :, :, d:d + 1], 1.0)
            qh_sb = persist.tile([P, nm, tok], bf16, tag="qh")
            kh_sb = persist.tile([P, nm, tok], bf16, tag="kh")

            # Streaming input DMAs round-robin over the SP and GpSimd
            # queues — a single queue moves ~100 GB/s, so the 1MB
            # activation chunks serialize into a 26us startup stall
            # otherwise.  (Only SP/Activation/GpSimd can initiate DMAs;
            # the Activation queue belongs to the pacing ScalarE and is
            # only borrowed for the prologue, before the exp stream.)
            # Steady-state streaming stays on SP: the GpSimd queue carries
            # the latency-critical partition_broadcast of every softmax
            # normalize, and a 1MB chunk costs ~5-10us of descriptor
            # generation that would stall it (measured as a net loss).
            prologue_q = [nc.sync, nc.gpsimd, nc.scalar]

            def next_q():
                if prologue_q:
                    return prologue_q.pop(0)
                return nc.sync

            def emit_qk_chunk(x_h, w_sb, b_sb, xh_sb, it, m, xtag):
                xt = xstream.tile([P, ncin, sck], bf16, tag=xtag,
                                  name=f"xt_{xtag}_{it}_{m}")
                next_q().dma_start(
                    out=xt,
                    in_=x_h[:, :].rearrange("(nb p) t -> p nb t", p=P)
                    [:, :, it * sck:(it + 1) * sck])
                ps = psum.tile([P, sck], fp32, tag="pp", bufs=2, name="psqk")
                for ci in range(ncin):
                    nc.tensor.matmul(
                        ps, w_sb[:, ci, m * P:(m + 1) * P], xt[:, ci, :],
                        start=(ci == 0), stop=(ci == ncin - 1))
                nc.vector.tensor_scalar(
                    out=xh_sb[:, m, it * sck:(it + 1) * sck],
                    in0=ps, scalar1=b_sb[:, m:m + 1], scalar2=None,
                    op0=mybir.AluOpType.add)

            def emit_v_tile(it):
                vt = vstream.tile([P, ncin, P], bf16, tag="vt",
                                  name=f"vt_{it}")
                next_q().dma_start(
                    out=vt,
                    in_=vT[:, :].rearrange("(nb p) t -> p nb t", p=P)
                    [:, :, it * P:(it + 1) * P])
                ps = psum.tile([P, cout], fp32, tag="pp", bufs=2, name="psv")
                for ci in range(ncin):
                    nc.tensor.matmul(ps, vt[:, ci, :], wv_sb[:, ci, :],
                                     start=(ci == 0), stop=(ci == ncin - 1))
                nc.vector.tensor_tensor(
                    out=vh_all[:, it, :, 0:d],
                    in0=ps.rearrange("p (h e) -> p h e", h=nh),
                    in1=bv_sb,
                    op=mybir.AluOpType.add)

            # Minimal projection prefix: only what the first scores matmul
            # of pair (tb0, hp0) needs — kh m0 tokens 0:sck and qh m0 tokens
            # 0:tqb.  Everything else is a filler inside the pair slots.
            emit_qk_chunk(kT, wk_sb, bk_sb, kh_sb, 0, 0, "xk")
            for it in range(tqb // sck):
                emit_qk_chunk(qT, wq_sb, bq_sb, qh_sb, it, 0, "xq")
            emit_late_consts()

            def mk_qk(x, it, m):
                if x == 'k':
                    return lambda: emit_qk_chunk(kT, wk_sb, bk_sb, kh_sb,
                                                 it, m, "xk")
                return lambda: emit_qk_chunk(qT, wq_sb, bq_sb, qh_sb,
                                             it, m, "xq")

            def mk_v(it):
                return lambda: emit_v_tile(it)

            # Per-(pair, slot) filler schedule.  Constraints: kh chunk
            # (it, m) before pair-of-m slot 4*it; qh chunks of a tq block
            # before the pair that consumes them; all V tiles before the
            # first PV sweep reaches them (pair1 slot 7 uses vh[14:16]).
            fill = {}

            def addf(p, s, th):
                fill.setdefault((p, s), []).append(th)

            full_sched = (nt == 16 and nm == 2 and nchunk == 4 and ntqb == 2
                          and nh == 4)
            if full_sched:
                addf(0, 1, mk_qk('k', 1, 0))
                addf(0, 3, mk_qk('k', 2, 0))
                addf(0, 5, mk_qk('k', 3, 0))
                addf(0, 7, mk_qk('q', 2, 0))
                addf(0, 9, mk_qk('k', 0, 1))
                addf(0, 11, mk_qk('q', 0, 1))
                addf(0, 13, mk_qk('q', 1, 1))
                for j, s in enumerate((2, 4, 6, 8, 10, 12, 14, 15)):
                    addf(0, s, mk_v(j))                    # V0..V7
                addf(1, 0, mk_qk('k', 1, 1))
                for j, s in enumerate((0, 1, 2, 3, 4, 5, 6, 6)):
                    addf(1, s, mk_v(8 + j))                # V8..V15
                addf(1, 7, mk_qk('k', 2, 1))
                addf(1, 9, mk_qk('q', 3, 0))
                addf(1, 11, mk_qk('k', 3, 1))
                addf(2, 1, mk_qk('q', 2, 1))
                addf(2, 3, mk_qk('q', 3, 1))
            else:
                # correctness fallback (small CoreSim configs): emit all
                # remaining chunks upfront; V tiles stream in pair 0.
                for m in range(nm):
                    for it in range(nchunk):
                        if m == 0 and it == 0:
                            continue
                        if m == 0 and it < tqb // sck:
                            mk_qk('k', it, 0)()
                            continue
                        mk_qk('k', it, m)()
                        mk_qk('q', it, m)()

            # ---- attention per head ---------------------------------------
            att_pair = [persist.tile([P, tok], bf16, tag=f"att{k}",
                                     name=f"att{k}")
                        for k in range(nko)]
            # Attention runs in head-pairs (even head on partitions 0:64,
            # odd on 64:128 — adjacent matmuls can row-tile concurrently on
            # the PE).  Phase 1 streams scores->exp into SBUF for the whole
            # pair (ScalarE stays saturated, nothing gates on PV); phase 2
            # does the PV accumulations at [65, sck] (one PSUM bank each)
            # and is interleaved, slot by slot, into the NEXT pair's phase 1
            # so it fills PE slack instead of stalling the exp stream.
            exp_bufs = 2 * nt + 6

            def emit_normalize(tb, m, h, p0, ck, stg):
                # reciprocal + GpSimd partition-broadcast + multiply.
                # NB: the custom-DVE reciprocal gets a partition-0 operand —
                # feeding it stg[64:65] directly breaks on hardware (passes
                # CoreSim), so copy the denominator row down first.
                den = smalls.tile([1, sck], fp32, tag="den",
                                  name=f"den_{tb}_{h}_{ck}")
                nc.vector.tensor_copy(out=den, in_=stg[d:d + 1, :])
                rec = smalls.tile([1, sck], fp32, tag="rec",
                                  name=f"rec_{tb}_{h}_{ck}")
                nc.vector.reciprocal_approx_fast(out=rec, in_=den)
                rep = smalls.tile([d, sck], fp32, tag="rep",
                                  name=f"rep_{tb}_{h}_{ck}")
                nc.gpsimd.partition_broadcast(out_ap=rep, in_ap=rec)
                c0 = tb * tqb + ck * sck
                nc.vector.tensor_tensor(
                    out=att_pair[m][p0:p0 + d, c0:c0 + sck],
                    in0=stg[0:d, :], in1=rep,
                    op=mybir.AluOpType.mult)

            def make_phase2_slots(tb, m, heads, exs):
                # Distribute the pair's PV work over nt emission slots,
                # ck-major when csk==2: first half of slots advances BOTH
                # heads' ck0 accumulators (2 ts tiles each), second half
                # ck1.  Each ck's normalize lands at mid-pair / pair-end,
                # so the downstream out-projection of that 512-token chunk
                # can start half a pair early instead of bursting at the
                # end.  Falls back to head-major for csk != 2.
                half = nt // 2
                state = {}

                def slot_ck(s):
                    ck = 0 if s < half else 1
                    if (s % half) == 0:
                        for hi, _ in enumerate(heads):
                            state[hi] = psum.tile(
                                [d + 1, sck], fp32, tag="pv", bufs=2,
                                name=f"pv_{tb}_{hi}_{ck}")
                    base = (s % half) * 2
                    for hi, (h, p0) in enumerate(heads):
                        for ts in (base, base + 1):
                            nc.tensor.matmul(
                                state[hi], vh_all[:, ts, h, :],
                                exs[(h, ts)][:, ck * sck:(ck + 1) * sck],
                                start=(ts == 0), stop=(ts == nt - 1))
                    if (s % half) == half - 1:
                        for hi, (h, p0) in enumerate(heads):
                            stg = smalls.tile([d + 1, sck], fp32, tag="stg",
                                              name=f"stg_{tb}_{h}_{ck}")
                            nc.vector.tensor_copy(out=stg, in_=state[hi])
                            emit_normalize(tb, m, h, p0, ck, stg)

                def slot_h(s):
                    h, p0 = heads[0] if s < half else heads[1]
                    if (s % half) == 0:
                        state[h] = [psum.tile([d + 1, sck], fp32, tag="pv",
                                              bufs=2,
                                              name=f"pv_{tb}_{h}_{ck}")
                                    for ck in range(csk)]
                    base = (s % half) * 2
                    for ck in range(csk):
                        for ts in (base, base + 1):
                            nc.tensor.matmul(
                                state[h][ck], vh_all[:, ts, h, :],
                                exs[(h, ts)][:, ck * sck:(ck + 1) * sck],
                                start=(ts == 0), stop=(ts == nt - 1))
                    if (s % half) == half - 1:
                        for ck in range(csk):
                            stg = smalls.tile([d + 1, sck], fp32, tag="stg",
                                              name=f"stg_{tb}_{h}_{ck}")
                            nc.vector.tensor_copy(out=stg, in_=state[h][ck])
                            emit_normalize(tb, m, h, p0, ck, stg)
                return slot_ck if (csk == 2 and nh == 4) else slot_h

            def emit_outproj_unit(tt, n, copy_eng):
                ps = psum.tile([P, ob], fp32, tag="pp", bufs=2, name="pso")
                for ko in range(nko):
                    nc.tensor.matmul(
                        ps, att_pair[ko][:, tt * P:(tt + 1) * P],
                        wo_sb[:, ko, n * ob:(n + 1) * ob],
                        start=(ko == 0), stop=(ko == nko - 1))
                o_sb = ostage.tile([P, ob], bf16, tag="ost")
                copy_eng(out=o_sb, in_=ps)
                nc.sync.dma_start(
                    out=outp[tt * P:(tt + 1) * P, n * ob:(n + 1) * ob],
                    in_=o_sb)

            def emit_outproj(tb, alternate=False):
                u = 0
                for tt in range(tb * (tqb // P), (tb + 1) * (tqb // P)):
                    for n in range(nob):
                        if alternate and u % 2 == 1:
                            emit_outproj_unit(tt, n, nc.scalar.copy)
                        else:
                            emit_outproj_unit(
                                tt, n,
                                lambda out, in_: nc.vector.tensor_copy(
                                    out=out, in_=in_))
                        u += 1

            pairs = [(tb, hp) for tb in range(ntqb) for hp in range(nh // 2)]
            last_idx = len(pairs) - 1
            pending = None        # (slot_fn, tb, was_last_in_tb, exs)
            self_pv = None        # [ck][hi] PSUM accumulators for last pair
            for idx, (tb, hp) in enumerate(pairs):
                m = hp if nm > 1 else 0
                heads = ((2 * hp, 0), (2 * hp + 1, d))
                is_last = (idx == last_idx and nt >= 16 and csk == 2
                           and not os.environ.get("K_NO_SELFPV"))
                exs = {}
                for i in range(nt):
                    for h, p0 in heads:
                        s_ps = psum.tile([P, tqb], fp32, tag="s",
                                         bufs=2, name="s_ps")
                        for cc in range(csk):
                            q0 = tb * tqb + cc * sck
                            nc.tensor.matmul(
                                s_ps[:, cc * sck:(cc + 1) * sck],
                                kh_sb[p0:p0 + d, m, i * P:(i + 1) * P],
                                qh_sb[p0:p0 + d, m, q0:q0 + sck],
                                start=True, stop=True)
                        ex = expool.tile([P, tqb], bf16, tag="ex",
                                         bufs=exp_bufs, name=f"ex_{h}_{i}")
                        nc.scalar.activation(
                            out=ex, in_=s_ps,
                            func=mybir.ActivationFunctionType.Exp,
                            scale=float(d) ** -0.5)
                        exs[(h, i)] = ex
                    for th in fill.get((idx, i), []):
                        th()
                    if not full_sched and idx == 0 and i < nt:
                        emit_v_tile(i)
                    if pending is not None and not os.environ.get(
                            "K_NO_INTERLEAVE"):
                        if is_last:
                            # compress the previous pair's drain into the
                            # first half so the final pair's own PV can
                            # self-interleave into the second half.
                            if i < nt // 2:
                                pending[0](2 * i)
                                pending[0](2 * i + 1)
                        else:
                            pending[0](i)
                    if (pending is not None and pending[2] and csk == 2
                            and not is_last and i >= nt // 2
                            and not os.environ.get("K_NO_INTERLEAVE")):
                        # ck0 of the previous pair's tb normalized at mid
                        # pair — spread that half of its out-projection
                        # here, one unit per slot (VectorE copies only;
                        # ScalarE is pacing the exp stream).
                        u = i - nt // 2
                        tt = pending[1] * (tqb // P) + u // nob
                        emit_outproj_unit(
                            tt, u % nob,
                            lambda out, in_: nc.vector.tensor_copy(
                                out=out, in_=in_))
                    if is_last and i >= nt // 2:
                        if i == nt // 2:
                            # ck0 accumulators reuse the freed "pv" bufs,
                            # ck1 the freed "pp" bufs (outproj burst done).
                            self_pv = [
                                [psum.tile([d + 1, sck], fp32,
                                           tag=("pv" if ck == 0 else "pp"),
                                           bufs=2, name=f"pvsi_{ck}_{h2}")
                                 for h2, _ in heads]
                                for ck in range(csk)]
                        for hi, (h2, _) in enumerate(heads):
                            for ts in (2 * (i - nt // 2),
                                       2 * (i - nt // 2) + 1):
                                for ck in range(csk):
                                    nc.tensor.matmul(
                                        self_pv[ck][hi],
                                        vh_all[:, ts, h2, :],
                                        exs[(h2, ts)][:,
                                                      ck * sck:(ck + 1) * sck],
                                        start=(ts == 0), stop=(ts == nt - 1))
                if pending is not None and pending[2]:
                    if csk == 2 and not os.environ.get("K_NO_INTERLEAVE"):
                        # ck0 units were spread into the slots above; emit
                        # only the ck1 half (normalized at pair end) here.
                        base = pending[1] * (tqb // P) + sck // P
                        u = 0
                        for tt in range(base, base + sck // P):
                            for n in range(nob):
                                if u % 2 == 1:
                                    emit_outproj_unit(tt, n, nc.scalar.copy)
                                else:
                                    emit_outproj_unit(
                                        tt, n,
                                        lambda out, in_:
                                        nc.vector.tensor_copy(
                                            out=out, in_=in_))
                                u += 1
                    else:
                        emit_outproj(pending[1], alternate=True)
                pending = (make_phase2_slots(tb, m, heads, exs), tb,
                           hp == nh // 2 - 1, exs)
                if os.environ.get("K_NO_INTERLEAVE") and idx != last_idx:
                    for s_i in range(nt):
                        pending[0](s_i)
            # Drain: all of the last pair's PV is already accumulated in
            # self_pv (both chunks), so the epilogue is normalize for every
            # (head, ck) first — releasing all self_pv PSUM (incl. the
            # pp-tagged ck1 tiles the out-proj needs) — then the final
            # out-projection with PSUM->SBUF copies alternating between
            # ScalarE (idle after the last exp) and VectorE.
            tb_l = pending[1]
            hp_l = nh // 2 - 1
            m_l = hp_l if nm > 1 else 0
            heads_l = ((2 * hp_l, 0), (2 * hp_l + 1, d))
            exs_l = pending[3]
            for ck in range(csk):
                for hi, (h, p0) in enumerate(heads_l):
                    if self_pv is not None:
                        pv = self_pv[ck][hi]
                    else:
                        pv = psum.tile([d + 1, sck], fp32, tag="pv", bufs=2,
                                       name=f"pvf_{h}_{ck}")
                        for ts in range(nt):
                            nc.tensor.matmul(
                                pv, vh_all[:, ts, h, :],
                                exs_l[(h, ts)][:, ck * sck:(ck + 1) * sck],
                                start=(ts == 0), stop=(ts == nt - 1))
                    stg = smalls.tile([d + 1, sck], fp32, tag="stg",
                                      name=f"stgf_{h}_{ck}")
                    nc.vector.tensor_copy(out=stg, in_=pv)
                    emit_normalize(tb_l, m_l, h, p0, ck, stg)
            u = 0
            for ck in range(csk):
                c0 = (tb_l * tqb + ck * sck) // P
                for tt in range(c0, c0 + sck // P):
                    for n in range(nob):
                        if u % 2 == 0:
                            emit_outproj_unit(tt, n, nc.scalar.copy)
                        else:
                            emit_outproj_unit(
                                tt, n,
                                lambda out, in_: nc.vector.tensor_copy(
                                    out=out, in_=in_))
                        u += 1

    nc.compile()
    return nc


def _host_inputs(q, k, v, Wq, Wk, Wv, Wo, bq, bk, bv,
                 tok=TOKENS, cin=C, cout=COUT, ngroup=NGROUP, ncores=NCORES):
    """Build per-core in_maps (host-side shard + transpose + bf16 cast)."""
    nm = max(1, cout // P)
    xT = {}
    for b in range(q.shape[0]):
        xT[('q', b)] = np.ascontiguousarray(q[b].T).astype(BF16)
        xT[('k', b)] = np.ascontiguousarray(k[b].T).astype(BF16)
        xT[('v', b)] = np.ascontiguousarray(v[b].T).astype(BF16)
    in_maps = []
    for core in range(ncores):
        b, g = core // ngroup, core % ngroup
        sl = slice(g * cout, (g + 1) * cout)
        in_maps.append({
            "qT": xT[('q', b)],
            "kT": xT[('k', b)],
            "vT": xT[('v', b)],
            "wqT": np.ascontiguousarray(Wq[sl, :].T).astype(BF16),
            "wkT": np.ascontiguousarray(Wk[sl, :].T).astype(BF16),
            "wvT": np.ascontiguousarray(Wv[sl, :].T).astype(BF16),
            "woT": np.ascontiguousarray(Wo[:, sl].T).astype(BF16),
            "bqv": np.ascontiguousarray(
                bq[sl].reshape(nm, P).T).astype(np.float32),
            "bkv": np.ascontiguousarray(
                bk[sl].reshape(nm, P).T).astype(np.float32),
            "bvv": np.ascontiguousarray(bv[sl][None, :]).astype(np.float32),
        })
    return in_maps


_NC_CACHE = {}


def _get_nc():
    if "nc" not in _NC_CACHE:
        _NC_CACHE["nc"] = build_nc()
    return _NC_CACHE["nc"]


def kernel(q, k, v, Wq, bq, Wk, bk, Wv, bv, Wo, bo):
    from concourse.bass_utils import run_bass_kernel_spmd

    q = np.asarray(q, dtype=np.float32)
    k = np.asarray(k, dtype=np.float32)
    v = np.asarray(v, dtype=np.float32)
    nc = _get_nc()
    in_maps = _host_inputs(q, k, v,
                           np.asarray(Wq, np.float32), np.asarray(Wk, np.float32),
                           np.asarray(Wv, np.float32), np.asarray(Wo, np.float32),
                           np.asarray(bq, np.float32), np.asarray(bk, np.float32),
                           np.asarray(bv, np.float32))
    res = run_bass_kernel_spmd(nc, in_maps, core_ids=list(range(NCORES)))
    parts = [np.asarray(r["outp"], dtype=np.float32) for r in res.results]
    out = np.stack(
        [sum(parts[b * NGROUP:(b + 1) * NGROUP]) for b in range(B)], axis=0)
    out = out + np.asarray(bo, np.float32)[None, None, :]
    return out.astype(np.float32)


# revision 20
# speedup vs baseline: 1.0261x; 1.0172x over previous
"""CrossAttention TRN2 kernel.

Full-input contract: kernel(**inputs) takes the unsharded numpy inputs of
  reference.py (q,k,v [2,2048,1024] fp32; Wq/Wk/Wv/Wo [1024,1024]; biases)
and returns the full [2,2048,1024] fp32 output.

Sharding: 8 cores = 2 batch groups x 4 head groups (tensor parallel over
heads).  Core c handles batch c//4 and heads [4*(c%4), 4*(c%4)+4).
Each core computes its heads' Q/K/V projections, attention, and a partial
output projection (row-slice of Wo); the host sums the 4 partials per batch
(no on-device collectives needed).

Per-core dataflow (all matmuls bf16 with fp32 PSUM accumulation):
  - host pre-transposes/casts activations (q^T,k^T,v^T [cin, tok] bf16) and
    weight slices, so contraction dims land on SBUF partitions directly.
  - scores are computed transposed ([ts, tq]) so the PV matmul can contract
    ts on partitions; a ones-column appended to vh yields the softmax
    denominator as PV row 64 for free.
  - exp runs on ScalarE (scale 1/sqrt(d) folded in), FD=1024 per activation.

Schedule (ScalarE exp is the pacing engine at ~1.34us per [128,1024] tile):
  attention pairs start right after the minimal projection prefix
  (K chunk 0 + Q chunks 0..1); all remaining projection chunks, the 16
  V-proj tiles and the tb0 out-projection are spread as per-slot fillers
  inside the pair iterations so the PE fills ScalarE-gated stalls without
  ever running far ahead.  The last pair self-interleaves its own PV for
  BOTH 512-column chunks so the epilogue is only normalize + out-proj.
"""

import os
import numpy as np
import ml_dtypes

BF16 = ml_dtypes.bfloat16

B, TOKENS, C = 2, 2048, 1024
NHEAD, D = 16, 64
NCORES = 8
NGROUP = 4                # head groups (cores per batch)
COUT = C // NGROUP        # 256 head-channels per core
NH = NHEAD // NGROUP      # 4 heads per core

P = 128                   # SBUF partitions


def build_nc(tok=TOKENS, cin=C, cout=COUT, nh=NH):
    """Emit the per-core Bass module. Parametric so a small version can be
    validated in CoreSim quickly. d=64 fixed; cout = nh*64."""
    import concourse.bacc as bacc
    import concourse.tile as tile
    import concourse.mybir as mybir

    d = D
    assert cout == nh * d
    ncin = cin // P               # cin tiles (contraction)
    nt = tok // P                 # token tiles
    nm = max(1, cout // P)        # 128-wide cout chunks (qhT/khT)
    tqb = min(1024, tok)          # tq block (exp FD)
    ntqb = tok // tqb
    sck = min(512, tok)           # matmul moving chunk
    csk = tqb // sck              # chunks per tq block
    nchunk = tok // sck           # qk chunks per m
    nob = max(1, min(2, cin // 512))  # out-proj cout chunks of 512
    ob = cin // nob               # out-proj N per chunk
    nko = cout // P if cout >= P else 1  # out-proj contraction tiles

    fp32 = mybir.dt.float32
    bf16 = mybir.dt.bfloat16

    nc = bacc.Bacc("TRN2", target_bir_lowering=False, debug=False)

    # Activations arrive pre-chunked in SBUF destination order
    # ([chunk, partition, cin-tile, token]) so every streaming DMA is 128
    # contiguous 8KB descriptors instead of 1024 strided 1KB ones — the
    # descriptor generation on the issuing queue was the startup gate.
    nchunk_io = tok // min(512, tok)
    qT = nc.dram_tensor("qT", [nchunk_io, P, cin // P, min(512, tok)],
                        bf16, kind="ExternalInput")
    kT = nc.dram_tensor("kT", [nchunk_io, P, cin // P, min(512, tok)],
                        bf16, kind="ExternalInput")
    vT = nc.dram_tensor("vT", [tok // P, P, cin // P, P],
                        bf16, kind="ExternalInput")
    wqT = nc.dram_tensor("wqT", [cin, cout], bf16, kind="ExternalInput")
    wkT = nc.dram_tensor("wkT", [cin, cout], bf16, kind="ExternalInput")
    wvT = nc.dram_tensor("wvT", [cin, cout], bf16, kind="ExternalInput")
    woT = nc.dram_tensor("woT", [cout, cin], bf16, kind="ExternalInput")
    bqv = nc.dram_tensor("bqv", [P, nm], fp32, kind="ExternalInput")
    bkv = nc.dram_tensor("bkv", [P, nm], fp32, kind="ExternalInput")
    bvv = nc.dram_tensor("bvv", [1, cout], fp32, kind="ExternalInput")
    outp = nc.dram_tensor("outp", [tok, cin], bf16, kind="ExternalOutput")

    with tile.TileContext(nc) as tc:
        from contextlib import ExitStack
        with ExitStack() as ctx:
            consts = ctx.enter_context(tc.tile_pool(name="consts", bufs=1))
            xstream = ctx.enter_context(tc.tile_pool(name="xstream", bufs=2))
            vstream = ctx.enter_context(tc.tile_pool(name="vstream", bufs=2))
            persist = ctx.enter_context(tc.tile_pool(name="persist", bufs=1))
            expool = ctx.enter_context(tc.tile_pool(name="expool", bufs=4))
            smalls = ctx.enter_context(tc.tile_pool(name="smalls", bufs=4))
            ostage = ctx.enter_context(tc.tile_pool(name="ostage", bufs=4))
            psum = ctx.enter_context(
                tc.tile_pool(name="psum", bufs=1, space="PSUM"))

            # ---- constants (K/Q weights first — they gate the first exp) ---
            wq_sb = consts.tile([P, ncin, cout], bf16, tag="wq")
            wk_sb = consts.tile([P, ncin, cout], bf16, tag="wk")
            wv_sb = consts.tile([P, ncin, cout], bf16, tag="wv")
            weng = nc.sync if os.environ.get("K_SYNC_CONSTS") else nc.scalar
            wengl = nc.sync if os.environ.get("K_SYNC_CONSTS") else nc.gpsimd
            for w_sb, w_h in ((wk_sb, wkT), (wq_sb, wqT)):
                weng.dma_start(
                    out=w_sb,
                    in_=w_h[:, :].rearrange("(nb p) co -> p nb co", p=P))
            bq_sb = consts.tile([P, nm], fp32, tag="bq")
            bk_sb = consts.tile([P, nm], fp32, tag="bk")
            nc.sync.dma_start(out=bq_sb, in_=bqv[:, :])
            nc.sync.dma_start(out=bk_sb, in_=bkv[:, :])
            # V/O weights + bv load behind the first Q/K activation chunks
            # (emitted below, scheduled after by SP queue order).
            wo_sb = consts.tile([P, nko, cin], bf16, tag="wo")
            bv_sb = consts.tile([P, nh, d], fp32, tag="bv")

            def emit_late_consts():
                wengl.dma_start(
                    out=wv_sb,
                    in_=wvT[:, :].rearrange("(nb p) co -> p nb co", p=P))
                wengl.dma_start(
                    out=wo_sb,
                    in_=woT[:, :].rearrange("(nb p) co -> p nb co", p=P))
                nc.gpsimd.dma_start(
                    out=bv_sb,
                    in_=bvv[:, :].rearrange("o (h e) -> o h e", h=nh)
                    .to_broadcast([P, nh, d]))

            # ---- projections ----------------------------------------------
            vh_all = persist.tile([P, nt, nh, d + 1], bf16, tag="vh")
            nc.vector.memset(vh_all[:, :, :, d:d + 1], 1.0)
            qh_sb = persist.tile([P, nm, tok], bf16, tag="qh")
            kh_sb = persist.tile([P, nm, tok], bf16, tag="kh")

            def emit_qk_chunk(x_h, w_sb, b_sb, xh_sb, it, m, xtag):
                xt = xstream.tile([P, ncin, sck], bf16, tag=xtag,
                                  name=f"xt_{xtag}_{it}_{m}")
                nc.sync.dma_start(out=xt, in_=x_h[it, :, :, :])
                ps = psum.tile([P, sck], fp32, tag="pp", bufs=2, name="psqk")
                for ci in range(ncin):
                    nc.tensor.matmul(
                        ps, w_sb[:, ci, m * P:(m + 1) * P], xt[:, ci, :],
                        start=(ci == 0), stop=(ci == ncin - 1))
                nc.vector.tensor_scalar(
                    out=xh_sb[:, m, it * sck:(it + 1) * sck],
                    in0=ps, scalar1=b_sb[:, m:m + 1], scalar2=None,
                    op0=mybir.AluOpType.add)

            def emit_v_tile(it):
                vt = vstream.tile([P, ncin, P], bf16, tag="vt",
                                  name=f"vt_{it}")
                nc.sync.dma_start(out=vt, in_=vT[it, :, :, :])
                ps = psum.tile([P, cout], fp32, tag="pp", bufs=2, name="psv")
                for ci in range(ncin):
                    nc.tensor.matmul(ps, vt[:, ci, :], wv_sb[:, ci, :],
                                     start=(ci == 0), stop=(ci == ncin - 1))
                nc.vector.tensor_tensor(
                    out=vh_all[:, it, :, 0:d],
                    in0=ps.rearrange("p (h e) -> p h e", h=nh),
                    in1=bv_sb,
                    op=mybir.AluOpType.add)

            # Minimal projection prefix: only what the first scores matmul
            # of pair (tb0, hp0) needs — kh m0 tokens 0:sck and qh m0 tokens
            # 0:tqb.  Everything else is a filler inside the pair slots.
            emit_qk_chunk(kT, wk_sb, bk_sb, kh_sb, 0, 0, "xk")
            for it in range(tqb // sck):
                emit_qk_chunk(qT, wq_sb, bq_sb, qh_sb, it, 0, "xq")
            emit_late_consts()

            def mk_qk(x, it, m):
                if x == 'k':
                    return lambda: emit_qk_chunk(kT, wk_sb, bk_sb, kh_sb,
                                                 it, m, "xk")
                return lambda: emit_qk_chunk(qT, wq_sb, bq_sb, qh_sb,
                                             it, m, "xq")

            def mk_v(it):
                return lambda: emit_v_tile(it)

            # Per-(pair, slot) filler schedule.  Constraints: kh chunk
            # (it, m) before pair-of-m slot 4*it; qh chunks of a tq block
            # before the pair that consumes them; all V tiles before the
            # first PV sweep reaches them (pair1 slot 7 uses vh[14:16]).
            fill = {}

            def addf(p, s, th):
                fill.setdefault((p, s), []).append(th)

            full_sched = (nt == 16 and nm == 2 and nchunk == 4 and ntqb == 2
                          and nh == 4)
            if full_sched:
                addf(0, 1, mk_qk('k', 1, 0))
                addf(0, 3, mk_qk('k', 2, 0))
                addf(0, 5, mk_qk('k', 3, 0))
                addf(0, 7, mk_qk('q', 2, 0))
                addf(0, 9, mk_qk('k', 0, 1))
                addf(0, 11, mk_qk('q', 0, 1))
                addf(0, 13, mk_qk('q', 1, 1))
                for j, s in enumerate((2, 4, 6, 8, 10, 12, 14, 15)):
                    addf(0, s, mk_v(j))                    # V0..V7
                addf(1, 0, mk_qk('k', 1, 1))
                for j, s in enumerate((0, 1, 2, 3, 4, 5, 6, 6)):
                    addf(1, s, mk_v(8 + j))                # V8..V15
                addf(1, 7, mk_qk('k', 2, 1))
                addf(1, 9, mk_qk('q', 3, 0))
                addf(1, 11, mk_qk('k', 3, 1))
                addf(2, 1, mk_qk('q', 2, 1))
                addf(2, 3, mk_qk('q', 3, 1))
            else:
                # correctness fallback (small CoreSim configs): emit all
                # remaining chunks upfront; V tiles stream in pair 0.
                for m in range(nm):
                    for it in range(nchunk):
                        if m == 0 and it == 0:
                            continue
                        if m == 0 and it < tqb // sck:
                            mk_qk('k', it, 0)()
                            continue
                        mk_qk('k', it, m)()
                        mk_qk('q', it, m)()

            # ---- attention per head ---------------------------------------
            att_pair = [persist.tile([P, tok], bf16, tag=f"att{k}",
                                     name=f"att{k}")
                        for k in range(nko)]
            # Attention runs in head-pairs (even head on partitions 0:64,
            # odd on 64:128 — adjacent matmuls can row-tile concurrently on
            # the PE).  Phase 1 streams scores->exp into SBUF for the whole
            # pair (ScalarE stays saturated, nothing gates on PV); phase 2
            # does the PV accumulations at [65, sck] (one PSUM bank each)
            # and is interleaved, slot by slot, into the NEXT pair's phase 1
            # so it fills PE slack instead of stalling the exp stream.
            exp_bufs = 2 * nt + 6

            def emit_normalize(tb, m, h, p0, ck, stg):
                # reciprocal + GpSimd partition-broadcast + multiply.
                # NB: the custom-DVE reciprocal gets a partition-0 operand —
                # feeding it stg[64:65] directly breaks on hardware (passes
                # CoreSim), so copy the denominator row down first.
                den = smalls.tile([1, sck], fp32, tag="den",
                                  name=f"den_{tb}_{h}_{ck}")
                nc.vector.tensor_copy(out=den, in_=stg[d:d + 1, :])
                rec = smalls.tile([1, sck], fp32, tag="rec",
                                  name=f"rec_{tb}_{h}_{ck}")
                nc.vector.reciprocal_approx_fast(out=rec, in_=den)
                rep = smalls.tile([d, sck], fp32, tag="rep",
                                  name=f"rep_{tb}_{h}_{ck}")
                nc.gpsimd.partition_broadcast(out_ap=rep, in_ap=rec)
                c0 = tb * tqb + ck * sck
                nc.vector.tensor_tensor(
                    out=att_pair[m][p0:p0 + d, c0:c0 + sck],
                    in0=stg[0:d, :], in1=rep,
                    op=mybir.AluOpType.mult)

            def make_phase2_slots(tb, m, heads, exs):
                # Distribute the pair's PV work over nt emission slots:
                # first half of slots = even head, second half = odd head;
                # each slot advances all csk chunk accumulators by 2 ts
                # tiles.  At the end of a head's slots, stage + normalize.
                half = nt // 2
                state = {}

                def slot(s):
                    h, p0 = heads[0] if s < half else heads[1]
                    if (s % half) == 0:
                        state[h] = [psum.tile([d + 1, sck], fp32, tag="pv",
                                              bufs=2,
                                              name=f"pv_{tb}_{h}_{ck}")
                                    for ck in range(csk)]
                    base = (s % half) * 2
                    for ck in range(csk):
                        for ts in (base, base + 1):
                            nc.tensor.matmul(
                                state[h][ck], vh_all[:, ts, h, :],
                                exs[(h, ts)][:, ck * sck:(ck + 1) * sck],
                                start=(ts == 0), stop=(ts == nt - 1))
                    if (s % half) == half - 1:
                        for ck in range(csk):
                            stg = smalls.tile([d + 1, sck], fp32, tag="stg",
                                              name=f"stg_{tb}_{h}_{ck}")
                            nc.vector.tensor_copy(out=stg, in_=state[h][ck])
                            emit_normalize(tb, m, h, p0, ck, stg)
                return slot

            def emit_outproj_unit(tt, n, copy_eng):
                ps = psum.tile([P, ob], fp32, tag="pp", bufs=2, name="pso")
                for ko in range(nko):
                    nc.tensor.matmul(
                        ps, att_pair[ko][:, tt * P:(tt + 1) * P],
                        wo_sb[:, ko, n * ob:(n + 1) * ob],
                        start=(ko == 0), stop=(ko == nko - 1))
                o_sb = ostage.tile([P, ob], bf16, tag="ost")
                copy_eng(out=o_sb, in_=ps)
                nc.sync.dma_start(
                    out=outp[tt * P:(tt + 1) * P, n * ob:(n + 1) * ob],
                    in_=o_sb)

            def emit_outproj(tb, alternate=False):
                u = 0
                for tt in range(tb * (tqb // P), (tb + 1) * (tqb // P)):
                    for n in range(nob):
                        if alternate and u % 2 == 1:
                            emit_outproj_unit(tt, n, nc.scalar.copy)
                        else:
                            emit_outproj_unit(
                                tt, n,
                                lambda out, in_: nc.vector.tensor_copy(
                                    out=out, in_=in_))
                        u += 1

            pairs = [(tb, hp) for tb in range(ntqb) for hp in range(nh // 2)]
            last_idx = len(pairs) - 1
            pending = None        # (slot_fn, tb, was_last_in_tb, exs)
            self_pv = None        # [ck][hi] PSUM accumulators for last pair
            for idx, (tb, hp) in enumerate(pairs):
                m = hp if nm > 1 else 0
                heads = ((2 * hp, 0), (2 * hp + 1, d))
                is_last = (idx == last_idx and nt >= 16 and csk == 2
                           and not os.environ.get("K_NO_SELFPV"))
                exs = {}
                for i in range(nt):
                    for h, p0 in heads:
                        s_ps = psum.tile([P, tqb], fp32, tag="s",
                                         bufs=2, name="s_ps")
                        for cc in range(csk):
                            q0 = tb * tqb + cc * sck
                            nc.tensor.matmul(
                                s_ps[:, cc * sck:(cc + 1) * sck],
                                kh_sb[p0:p0 + d, m, i * P:(i + 1) * P],
                                qh_sb[p0:p0 + d, m, q0:q0 + sck],
                                start=True, stop=True)
                        ex = expool.tile([P, tqb], bf16, tag="ex",
                                         bufs=exp_bufs, name=f"ex_{h}_{i}")
                        nc.scalar.activation(
                            out=ex, in_=s_ps,
                            func=mybir.ActivationFunctionType.Exp,
                            scale=float(d) ** -0.5)
                        exs[(h, i)] = ex
                    for th in fill.get((idx, i), []):
                        th()
                    if not full_sched and idx == 0 and i < nt:
                        emit_v_tile(i)
                    if pending is not None and not os.environ.get(
                            "K_NO_INTERLEAVE"):
                        if is_last:
                            # compress the previous pair's drain into the
                            # first half so the final pair's own PV can
                            # self-interleave into the second half.
                            if i < nt // 2:
                                pending[0](2 * i)
                                pending[0](2 * i + 1)
                        else:
                            pending[0](i)
                    if is_last and i >= nt // 2:
                        if i == nt // 2:
                            # ck0 accumulators reuse the freed "pv" bufs,
                            # ck1 the freed "pp" bufs (outproj burst done).
                            self_pv = [
                                [psum.tile([d + 1, sck], fp32,
                                           tag=("pv" if ck == 0 else "pp"),
                                           bufs=2, name=f"pvsi_{ck}_{h2}")
                                 for h2, _ in heads]
                                for ck in range(csk)]
                        for hi, (h2, _) in enumerate(heads):
                            for ts in (2 * (i - nt // 2),
                                       2 * (i - nt // 2) + 1):
                                for ck in range(csk):
                                    nc.tensor.matmul(
                                        self_pv[ck][hi],
                                        vh_all[:, ts, h2, :],
                                        exs[(h2, ts)][:,
                                                      ck * sck:(ck + 1) * sck],
                                        start=(ts == 0), stop=(ts == nt - 1))
                if pending is not None and pending[2]:
                    emit_outproj(pending[1], alternate=True)
                pending = (make_phase2_slots(tb, m, heads, exs), tb,
                           hp == nh // 2 - 1, exs)
                if os.environ.get("K_NO_INTERLEAVE") and idx != last_idx:
                    for s_i in range(nt):
                        pending[0](s_i)
            # Drain: all of the last pair's PV is already accumulated in
            # self_pv (both chunks), so the epilogue is normalize for every
            # (head, ck) first — releasing all self_pv PSUM (incl. the
            # pp-tagged ck1 tiles the out-proj needs) — then the final
            # out-projection with PSUM->SBUF copies alternating between
            # ScalarE (idle after the last exp) and VectorE.
            tb_l = pending[1]
            hp_l = nh // 2 - 1
            m_l = hp_l if nm > 1 else 0
            heads_l = ((2 * hp_l, 0), (2 * hp_l + 1, d))
            exs_l = pending[3]
            for ck in range(csk):
                for hi, (h, p0) in enumerate(heads_l):
                    if self_pv is not None:
                        pv = self_pv[ck][hi]
                    else:
                        pv = psum.tile([d + 1, sck], fp32, tag="pv", bufs=2,
                                       name=f"pvf_{h}_{ck}")
                        for ts in range(nt):
                            nc.tensor.matmul(
                                pv, vh_all[:, ts, h, :],
                                exs_l[(h, ts)][:, ck * sck:(ck + 1) * sck],
                                start=(ts == 0), stop=(ts == nt - 1))
                    stg = smalls.tile([d + 1, sck], fp32, tag="stg",
                                      name=f"stgf_{h}_{ck}")
                    nc.vector.tensor_copy(out=stg, in_=pv)
                    emit_normalize(tb_l, m_l, h, p0, ck, stg)
            u = 0
            for ck in range(csk):
                c0 = (tb_l * tqb + ck * sck) // P
                for tt in range(c0, c0 + sck // P):
                    for n in range(nob):
                        if u % 2 == 0:
                            emit_outproj_unit(tt, n, nc.scalar.copy)
                        else:
                            emit_outproj_unit(
                                tt, n,
                                lambda out, in_: nc.vector.tensor_copy(
                                    out=out, in_=in_))
                        u += 1

    nc.compile()
    return nc


def _host_inputs(q, k, v, Wq, Wk, Wv, Wo, bq, bk, bv,
                 tok=TOKENS, cin=C, cout=COUT, ngroup=NGROUP, ncores=NCORES):
    """Build per-core in_maps (host-side shard + transpose + bf16 cast)."""
    nm = max(1, cout // P)
    ncin = cin // P
    sck = min(512, tok)
    nchunk = tok // sck
    nt = tok // P

    def chunked(x, width, n):
        # [tok, cin] -> [n, P, ncin, width]: (c, p, nb, t) = x[c*w+t, nb*P+p]
        xt = np.ascontiguousarray(x.T).astype(BF16)          # [cin, tok]
        return np.ascontiguousarray(
            xt.reshape(ncin, P, n, width).transpose(2, 1, 0, 3))

    xT = {}
    for b in range(q.shape[0]):
        xT[('q', b)] = chunked(q[b], sck, nchunk)
        xT[('k', b)] = chunked(k[b], sck, nchunk)
        xT[('v', b)] = chunked(v[b], P, nt)
    in_maps = []
    for core in range(ncores):
        b, g = core // ngroup, core % ngroup
        sl = slice(g * cout, (g + 1) * cout)
        in_maps.append({
            "qT": xT[('q', b)],
            "kT": xT[('k', b)],
            "vT": xT[('v', b)],
            "wqT": np.ascontiguousarray(Wq[sl, :].T).astype(BF16),
            "wkT": np.ascontiguousarray(Wk[sl, :].T).astype(BF16),
            "wvT": np.ascontiguousarray(Wv[sl, :].T).astype(BF16),
            "woT": np.ascontiguousarray(Wo[:, sl].T).astype(BF16),
            "bqv": np.ascontiguousarray(
                bq[sl].reshape(nm, P).T).astype(np.float32),
            "bkv": np.ascontiguousarray(
                bk[sl].reshape(nm, P).T).astype(np.float32),
            "bvv": np.ascontiguousarray(bv[sl][None, :]).astype(np.float32),
        })
    return in_maps


_NC_CACHE = {}


def _get_nc():
    if "nc" not in _NC_CACHE:
        _NC_CACHE["nc"] = build_nc()
    return _NC_CACHE["nc"]


def kernel(q, k, v, Wq, bq, Wk, bk, Wv, bv, Wo, bo):
    from concourse.bass_utils import run_bass_kernel_spmd

    q = np.asarray(q, dtype=np.float32)
    k = np.asarray(k, dtype=np.float32)
    v = np.asarray(v, dtype=np.float32)
    nc = _get_nc()
    in_maps = _host_inputs(q, k, v,
                           np.asarray(Wq, np.float32), np.asarray(Wk, np.float32),
                           np.asarray(Wv, np.float32), np.asarray(Wo, np.float32),
                           np.asarray(bq, np.float32), np.asarray(bk, np.float32),
                           np.asarray(bv, np.float32))
    res = run_bass_kernel_spmd(nc, in_maps, core_ids=list(range(NCORES)))
    parts = [np.asarray(r["outp"], dtype=np.float32) for r in res.results]
    out = np.stack(
        [sum(parts[b * NGROUP:(b + 1) * NGROUP]) for b in range(B)], axis=0)
    out = out + np.asarray(bo, np.float32)[None, None, :]
    return out.astype(np.float32)


# revision 21
# speedup vs baseline: 1.0689x; 1.0418x over previous
"""CrossAttention TRN2 kernel.

Full-input contract: kernel(**inputs) takes the unsharded numpy inputs of
  reference.py (q,k,v [2,2048,1024] fp32; Wq/Wk/Wv/Wo [1024,1024]; biases)
and returns the full [2,2048,1024] fp32 output.

Sharding: 8 cores = 2 batch groups x 4 head groups (tensor parallel over
heads).  Core c handles batch c//4 and heads [4*(c%4), 4*(c%4)+4).
Each core computes its heads' Q/K/V projections, attention, and a partial
output projection (row-slice of Wo); the host sums the 4 partials per batch
(no on-device collectives needed).

Per-core dataflow (all matmuls bf16 with fp32 PSUM accumulation):
  - host pre-transposes/casts activations (q^T,k^T,v^T [cin, tok] bf16) and
    weight slices, so contraction dims land on SBUF partitions directly.
  - scores are computed transposed ([ts, tq]) so the PV matmul can contract
    ts on partitions; a ones-column appended to vh yields the softmax
    denominator as PV row 64 for free.
  - exp runs on ScalarE (scale 1/sqrt(d) folded in), FD=1024 per activation.

Schedule (ScalarE exp is the pacing engine at ~1.34us per [128,1024] tile):
  attention pairs start right after the minimal projection prefix
  (K chunk 0 + Q chunks 0..1); all remaining projection chunks, the 16
  V-proj tiles and the tb0 out-projection are spread as per-slot fillers
  inside the pair iterations so the PE fills ScalarE-gated stalls without
  ever running far ahead.  The last pair self-interleaves its own PV for
  BOTH 512-column chunks so the epilogue is only normalize + out-proj.
"""

import os
import numpy as np
import ml_dtypes

BF16 = ml_dtypes.bfloat16

B, TOKENS, C = 2, 2048, 1024
NHEAD, D = 16, 64
NCORES = 8
NGROUP = 4                # head groups (cores per batch)
COUT = C // NGROUP        # 256 head-channels per core
NH = NHEAD // NGROUP      # 4 heads per core

P = 128                   # SBUF partitions


def build_nc(tok=TOKENS, cin=C, cout=COUT, nh=NH):
    """Emit the per-core Bass module. Parametric so a small version can be
    validated in CoreSim quickly. d=64 fixed; cout = nh*64."""
    import concourse.bacc as bacc
    import concourse.tile as tile
    import concourse.mybir as mybir

    d = D
    assert cout == nh * d
    ncin = cin // P               # cin tiles (contraction)
    nt = tok // P                 # token tiles
    nm = max(1, cout // P)        # 128-wide cout chunks (qhT/khT)
    tqb = min(1024, tok)          # tq block (exp FD)
    ntqb = tok // tqb
    sck = min(512, tok)           # matmul moving chunk
    csk = tqb // sck              # chunks per tq block
    nchunk = tok // sck           # qk chunks per m
    nob = max(1, min(2, cin // 512))  # out-proj cout chunks of 512
    ob = cin // nob               # out-proj N per chunk
    nko = cout // P if cout >= P else 1  # out-proj contraction tiles

    fp32 = mybir.dt.float32
    bf16 = mybir.dt.bfloat16

    nc = bacc.Bacc("TRN2", target_bir_lowering=False, debug=False)

    qT = nc.dram_tensor("qT", [cin, tok], bf16, kind="ExternalInput")
    kT = nc.dram_tensor("kT", [cin, tok], bf16, kind="ExternalInput")
    vT = nc.dram_tensor("vT", [cin, tok], bf16, kind="ExternalInput")
    wqT = nc.dram_tensor("wqT", [cin, cout], bf16, kind="ExternalInput")
    wkT = nc.dram_tensor("wkT", [cin, cout], bf16, kind="ExternalInput")
    wvT = nc.dram_tensor("wvT", [cin, cout], bf16, kind="ExternalInput")
    woT = nc.dram_tensor("woT", [cout, cin], bf16, kind="ExternalInput")
    bqv = nc.dram_tensor("bqv", [P, nm], fp32, kind="ExternalInput")
    bkv = nc.dram_tensor("bkv", [P, nm], fp32, kind="ExternalInput")
    bvv = nc.dram_tensor("bvv", [1, cout], fp32, kind="ExternalInput")
    outp = nc.dram_tensor("outp", [tok, cin], bf16, kind="ExternalOutput")

    with tile.TileContext(nc) as tc:
        from contextlib import ExitStack
        with ExitStack() as ctx:
            consts = ctx.enter_context(tc.tile_pool(name="consts", bufs=1))
            xstream = ctx.enter_context(tc.tile_pool(name="xstream", bufs=2))
            vstream = ctx.enter_context(tc.tile_pool(name="vstream", bufs=2))
            persist = ctx.enter_context(tc.tile_pool(name="persist", bufs=1))
            expool = ctx.enter_context(tc.tile_pool(name="expool", bufs=4))
            smalls = ctx.enter_context(tc.tile_pool(name="smalls", bufs=4))
            ostage = ctx.enter_context(tc.tile_pool(name="ostage", bufs=4))
            psum = ctx.enter_context(
                tc.tile_pool(name="psum", bufs=1, space="PSUM"))

            # ---- constants (K/Q weights first — they gate the first exp) ---
            wq_sb = consts.tile([P, ncin, cout], bf16, tag="wq")
            wk_sb = consts.tile([P, ncin, cout], bf16, tag="wk")
            wv_sb = consts.tile([P, ncin, cout], bf16, tag="wv")
            weng = nc.sync if os.environ.get("K_SYNC_CONSTS") else nc.scalar
            wengl = nc.sync if os.environ.get("K_SYNC_CONSTS") else nc.gpsimd
            for w_sb, w_h in ((wk_sb, wkT), (wq_sb, wqT)):
                weng.dma_start(
                    out=w_sb,
                    in_=w_h[:, :].rearrange("(nb p) co -> p nb co", p=P))
            bq_sb = consts.tile([P, nm], fp32, tag="bq")
            bk_sb = consts.tile([P, nm], fp32, tag="bk")
            nc.sync.dma_start(out=bq_sb, in_=bqv[:, :])
            nc.sync.dma_start(out=bk_sb, in_=bkv[:, :])
            # V/O weights + bv load behind the first Q/K activation chunks
            # (emitted below, scheduled after by SP queue order).
            wo_sb = consts.tile([P, nko, cin], bf16, tag="wo")
            bv_sb = consts.tile([P, nh, d], fp32, tag="bv")

            def emit_late_consts():
                wengl.dma_start(
                    out=wv_sb,
                    in_=wvT[:, :].rearrange("(nb p) co -> p nb co", p=P))
                wengl.dma_start(
                    out=wo_sb,
                    in_=woT[:, :].rearrange("(nb p) co -> p nb co", p=P))
                nc.gpsimd.dma_start(
                    out=bv_sb,
                    in_=bvv[:, :].rearrange("o (h e) -> o h e", h=nh)
                    .to_broadcast([P, nh, d]))

            # ---- projections ----------------------------------------------
            vh_all = persist.tile([P, nt, nh, d + 1], bf16, tag="vh")
            nc.vector.memset(vh_all[:, :, :, d:d + 1], 1.0)
            qh_sb = persist.tile([P, nm, tok], bf16, tag="qh")
            kh_sb = persist.tile([P, nm, tok], bf16, tag="kh")

            def emit_qk_chunk(x_h, w_sb, b_sb, xh_sb, it, m, xtag):
                xt = xstream.tile([P, ncin, sck], bf16, tag=xtag,
                                  name=f"xt_{xtag}_{it}_{m}")
                nc.sync.dma_start(
                    out=xt,
                    in_=x_h[:, :].rearrange("(nb p) t -> p nb t", p=P)
                    [:, :, it * sck:(it + 1) * sck])
                ps = psum.tile([P, sck], fp32, tag="pp", bufs=2, name="psqk")
                for ci in range(ncin):
                    nc.tensor.matmul(
                        ps, w_sb[:, ci, m * P:(m + 1) * P], xt[:, ci, :],
                        start=(ci == 0), stop=(ci == ncin - 1))
                nc.vector.tensor_scalar(
                    out=xh_sb[:, m, it * sck:(it + 1) * sck],
                    in0=ps, scalar1=b_sb[:, m:m + 1], scalar2=None,
                    op0=mybir.AluOpType.add)

            def emit_v_tile(it):
                vt = vstream.tile([P, ncin, P], bf16, tag="vt",
                                  name=f"vt_{it}")
                nc.sync.dma_start(
                    out=vt,
                    in_=vT[:, :].rearrange("(nb p) t -> p nb t", p=P)
                    [:, :, it * P:(it + 1) * P])
                ps = psum.tile([P, cout], fp32, tag="pp", bufs=2, name="psv")
                for ci in range(ncin):
                    nc.tensor.matmul(ps, vt[:, ci, :], wv_sb[:, ci, :],
                                     start=(ci == 0), stop=(ci == ncin - 1))
                nc.vector.tensor_tensor(
                    out=vh_all[:, it, :, 0:d],
                    in0=ps.rearrange("p (h e) -> p h e", h=nh),
                    in1=bv_sb,
                    op=mybir.AluOpType.add)

            # Minimal projection prefix: only what the first scores matmul
            # of pair (tb0, hp0) needs — kh m0 tokens 0:sck and qh m0 tokens
            # 0:tqb.  Everything else is a filler inside the pair slots.
            emit_qk_chunk(kT, wk_sb, bk_sb, kh_sb, 0, 0, "xk")
            for it in range(tqb // sck):
                emit_qk_chunk(qT, wq_sb, bq_sb, qh_sb, it, 0, "xq")
            emit_late_consts()

            def mk_qk(x, it, m):
                if x == 'k':
                    return lambda: emit_qk_chunk(kT, wk_sb, bk_sb, kh_sb,
                                                 it, m, "xk")
                return lambda: emit_qk_chunk(qT, wq_sb, bq_sb, qh_sb,
                                             it, m, "xq")

            def mk_v(it):
                return lambda: emit_v_tile(it)

            # Per-(pair, slot) filler schedule.  Constraints: kh chunk
            # (it, m) before pair-of-m slot 4*it; qh chunks of a tq block
            # before the pair that consumes them; all V tiles before the
            # first PV sweep reaches them (pair1 slot 7 uses vh[14:16]).
            fill = {}

            def addf(p, s, th):
                fill.setdefault((p, s), []).append(th)

            full_sched = (nt == 16 and nm == 2 and nchunk == 4 and ntqb == 2
                          and nh == 4)
            if full_sched:
                addf(0, 1, mk_qk('k', 1, 0))
                addf(0, 3, mk_qk('k', 2, 0))
                addf(0, 5, mk_qk('k', 3, 0))
                addf(0, 7, mk_qk('q', 2, 0))
                addf(0, 9, mk_qk('k', 0, 1))
                addf(0, 11, mk_qk('q', 0, 1))
                addf(0, 13, mk_qk('q', 1, 1))
                for j, s in enumerate((2, 4, 6, 8, 10, 12, 14, 15)):
                    addf(0, s, mk_v(j))                    # V0..V7
                addf(1, 0, mk_qk('k', 1, 1))
                for j, s in enumerate((0, 1, 2, 3, 4, 5, 6, 6)):
                    addf(1, s, mk_v(8 + j))                # V8..V15
                addf(1, 7, mk_qk('k', 2, 1))
                addf(1, 9, mk_qk('q', 3, 0))
                addf(1, 11, mk_qk('k', 3, 1))
                addf(2, 1, mk_qk('q', 2, 1))
                addf(2, 3, mk_qk('q', 3, 1))
            else:
                # correctness fallback (small CoreSim configs): emit all
                # remaining chunks upfront; V tiles stream in pair 0.
                for m in range(nm):
                    for it in range(nchunk):
                        if m == 0 and it == 0:
                            continue
                        if m == 0 and it < tqb // sck:
                            mk_qk('k', it, 0)()
                            continue
                        mk_qk('k', it, m)()
                        mk_qk('q', it, m)()

            # ---- attention per head ---------------------------------------
            att_pair = [persist.tile([P, tok], bf16, tag=f"att{k}",
                                     name=f"att{k}")
                        for k in range(nko)]
            # Attention runs in head-pairs (even head on partitions 0:64,
            # odd on 64:128 — adjacent matmuls can row-tile concurrently on
            # the PE).  Phase 1 streams scores->exp into SBUF for the whole
            # pair (ScalarE stays saturated, nothing gates on PV); phase 2
            # does the PV accumulations at [65, sck] (one PSUM bank each)
            # and is interleaved, slot by slot, into the NEXT pair's phase 1
            # so it fills PE slack instead of stalling the exp stream.
            exp_bufs = 2 * nt + 6

            def emit_normalize(tb, m, h, p0, ck, stg):
                # reciprocal + GpSimd partition-broadcast + multiply.
                # NB: the custom-DVE reciprocal gets a partition-0 operand —
                # feeding it stg[64:65] directly breaks on hardware (passes
                # CoreSim), so copy the denominator row down first.
                den = smalls.tile([1, sck], fp32, tag="den",
                                  name=f"den_{tb}_{h}_{ck}")
                nc.vector.tensor_copy(out=den, in_=stg[d:d + 1, :])
                rec = smalls.tile([1, sck], fp32, tag="rec",
                                  name=f"rec_{tb}_{h}_{ck}")
                nc.vector.reciprocal_approx_fast(out=rec, in_=den)
                rep = smalls.tile([d, sck], fp32, tag="rep",
                                  name=f"rep_{tb}_{h}_{ck}")
                nc.gpsimd.partition_broadcast(out_ap=rep, in_ap=rec)
                c0 = tb * tqb + ck * sck
                nc.vector.tensor_tensor(
                    out=att_pair[m][p0:p0 + d, c0:c0 + sck],
                    in0=stg[0:d, :], in1=rep,
                    op=mybir.AluOpType.mult)

            def make_phase2_slots(tb, m, heads, exs):
                # Distribute the pair's PV work over nt emission slots:
                # first half of slots = even head, second half = odd head;
                # each slot advances all csk chunk accumulators by 2 ts
                # tiles.  At the end of a head's slots, stage + normalize.
                half = nt // 2
                state = {}

                def slot(s):
                    h, p0 = heads[0] if s < half else heads[1]
                    if (s % half) == 0:
                        state[h] = [psum.tile([d + 1, sck], fp32, tag="pv",
                                              bufs=2,
                                              name=f"pv_{tb}_{h}_{ck}")
                                    for ck in range(csk)]
                    base = (s % half) * 2
                    for ck in range(csk):
                        for ts in (base, base + 1):
                            nc.tensor.matmul(
                                state[h][ck], vh_all[:, ts, h, :],
                                exs[(h, ts)][:, ck * sck:(ck + 1) * sck],
                                start=(ts == 0), stop=(ts == nt - 1))
                    if (s % half) == half - 1:
                        for ck in range(csk):
                            stg = smalls.tile([d + 1, sck], fp32, tag="stg",
                                              name=f"stg_{tb}_{h}_{ck}")
                            nc.vector.tensor_copy(out=stg, in_=state[h][ck])
                            emit_normalize(tb, m, h, p0, ck, stg)
                return slot

            def emit_outproj_unit(tt, n, copy_eng):
                ps = psum.tile([P, ob], fp32, tag="pp", bufs=2, name="pso")
                for ko in range(nko):
                    nc.tensor.matmul(
                        ps, att_pair[ko][:, tt * P:(tt + 1) * P],
                        wo_sb[:, ko, n * ob:(n + 1) * ob],
                        start=(ko == 0), stop=(ko == nko - 1))
                o_sb = ostage.tile([P, ob], bf16, tag="ost")
                copy_eng(out=o_sb, in_=ps)
                nc.sync.dma_start(
                    out=outp[tt * P:(tt + 1) * P, n * ob:(n + 1) * ob],
                    in_=o_sb)

            def emit_outproj(tb, alternate=False):
                u = 0
                for tt in range(tb * (tqb // P), (tb + 1) * (tqb // P)):
                    for n in range(nob):
                        if alternate and u % 2 == 1:
                            emit_outproj_unit(tt, n, nc.scalar.copy)
                        else:
                            emit_outproj_unit(
                                tt, n,
                                lambda out, in_: nc.vector.tensor_copy(
                                    out=out, in_=in_))
                        u += 1

            pairs = [(tb, hp) for tb in range(ntqb) for hp in range(nh // 2)]
            last_idx = len(pairs) - 1
            pending = None        # (slot_fn, tb, was_last_in_tb, exs)
            self_pv = None        # [ck][hi] PSUM accumulators for last pair
            for idx, (tb, hp) in enumerate(pairs):
                m = hp if nm > 1 else 0
                heads = ((2 * hp, 0), (2 * hp + 1, d))
                is_last = (idx == last_idx and nt >= 16 and csk == 2
                           and not os.environ.get("K_NO_SELFPV"))
                exs = {}
                for i in range(nt):
                    for h, p0 in heads:
                        s_ps = psum.tile([P, tqb], fp32, tag="s",
                                         bufs=2, name="s_ps")
                        for cc in range(csk):
                            q0 = tb * tqb + cc * sck
                            nc.tensor.matmul(
                                s_ps[:, cc * sck:(cc + 1) * sck],
                                kh_sb[p0:p0 + d, m, i * P:(i + 1) * P],
                                qh_sb[p0:p0 + d, m, q0:q0 + sck],
                                start=True, stop=True)
                        ex = expool.tile([P, tqb], bf16, tag="ex",
                                         bufs=exp_bufs, name=f"ex_{h}_{i}")
                        nc.scalar.activation(
                            out=ex, in_=s_ps,
                            func=mybir.ActivationFunctionType.Exp,
                            scale=float(d) ** -0.5)
                        exs[(h, i)] = ex
                    for th in fill.get((idx, i), []):
                        th()
                    if not full_sched and idx == 0 and i < nt:
                        emit_v_tile(i)
                    if pending is not None and not os.environ.get(
                            "K_NO_INTERLEAVE"):
                        if is_last:
                            # compress the previous pair's drain into the
                            # first half so the final pair's own PV can
                            # self-interleave into the second half.
                            if i < nt // 2:
                                pending[0](2 * i)
                                pending[0](2 * i + 1)
                        else:
                            pending[0](i)
                    if is_last and i >= nt // 2:
                        if i == nt // 2:
                            # ck0 accumulators reuse the freed "pv" bufs,
                            # ck1 the freed "pp" bufs (outproj burst done).
                            self_pv = [
                                [psum.tile([d + 1, sck], fp32,
                                           tag=("pv" if ck == 0 else "pp"),
                                           bufs=2, name=f"pvsi_{ck}_{h2}")
                                 for h2, _ in heads]
                                for ck in range(csk)]
                        for hi, (h2, _) in enumerate(heads):
                            for ts in (2 * (i - nt // 2),
                                       2 * (i - nt // 2) + 1):
                                for ck in range(csk):
                                    nc.tensor.matmul(
                                        self_pv[ck][hi],
                                        vh_all[:, ts, h2, :],
                                        exs[(h2, ts)][:,
                                                      ck * sck:(ck + 1) * sck],
                                        start=(ts == 0), stop=(ts == nt - 1))
                if pending is not None and pending[2]:
                    emit_outproj(pending[1], alternate=True)
                pending = (make_phase2_slots(tb, m, heads, exs), tb,
                           hp == nh // 2 - 1, exs)
                if os.environ.get("K_NO_INTERLEAVE") and idx != last_idx:
                    for s_i in range(nt):
                        pending[0](s_i)
            # Drain: all of the last pair's PV is already accumulated in
            # self_pv (both chunks), so the epilogue is normalize for every
            # (head, ck) first — releasing all self_pv PSUM (incl. the
            # pp-tagged ck1 tiles the out-proj needs) — then the final
            # out-projection with PSUM->SBUF copies alternating between
            # ScalarE (idle after the last exp) and VectorE.
            tb_l = pending[1]
            hp_l = nh // 2 - 1
            m_l = hp_l if nm > 1 else 0
            heads_l = ((2 * hp_l, 0), (2 * hp_l + 1, d))
            exs_l = pending[3]
            for ck in range(csk):
                for hi, (h, p0) in enumerate(heads_l):
                    if self_pv is not None:
                        pv = self_pv[ck][hi]
                    else:
                        pv = psum.tile([d + 1, sck], fp32, tag="pv", bufs=2,
                                       name=f"pvf_{h}_{ck}")
                        for ts in range(nt):
                            nc.tensor.matmul(
                                pv, vh_all[:, ts, h, :],
                                exs_l[(h, ts)][:, ck * sck:(ck + 1) * sck],
                                start=(ts == 0), stop=(ts == nt - 1))
                    stg = smalls.tile([d + 1, sck], fp32, tag="stg",
                                      name=f"stgf_{h}_{ck}")
                    nc.vector.tensor_copy(out=stg, in_=pv)
                    emit_normalize(tb_l, m_l, h, p0, ck, stg)
            u = 0
            for ck in range(csk):
                c0 = (tb_l * tqb + ck * sck) // P
                for tt in range(c0, c0 + sck // P):
                    for n in range(nob):
                        if u % 2 == 0:
                            emit_outproj_unit(tt, n, nc.scalar.copy)
                        else:
                            emit_outproj_unit(
                                tt, n,
                                lambda out, in_: nc.vector.tensor_copy(
                                    out=out, in_=in_))
                        u += 1

    nc.compile()
    return nc


def _host_inputs(q, k, v, Wq, Wk, Wv, Wo, bq, bk, bv,
                 tok=TOKENS, cin=C, cout=COUT, ngroup=NGROUP, ncores=NCORES):
    """Build per-core in_maps (host-side shard + transpose + bf16 cast)."""
    nm = max(1, cout // P)
    xT = {}
    for b in range(q.shape[0]):
        xT[('q', b)] = np.ascontiguousarray(q[b].T).astype(BF16)
        xT[('k', b)] = np.ascontiguousarray(k[b].T).astype(BF16)
        xT[('v', b)] = np.ascontiguousarray(v[b].T).astype(BF16)
    in_maps = []
    for core in range(ncores):
        b, g = core // ngroup, core % ngroup
        sl = slice(g * cout, (g + 1) * cout)
        in_maps.append({
            "qT": xT[('q', b)],
            "kT": xT[('k', b)],
            "vT": xT[('v', b)],
            "wqT": np.ascontiguousarray(Wq[sl, :].T).astype(BF16),
            "wkT": np.ascontiguousarray(Wk[sl, :].T).astype(BF16),
            "wvT": np.ascontiguousarray(Wv[sl, :].T).astype(BF16),
            "woT": np.ascontiguousarray(Wo[:, sl].T).astype(BF16),
            "bqv": np.ascontiguousarray(
                bq[sl].reshape(nm, P).T).astype(np.float32),
            "bkv": np.ascontiguousarray(
                bk[sl].reshape(nm, P).T).astype(np.float32),
            "bvv": np.ascontiguousarray(bv[sl][None, :]).astype(np.float32),
        })
    return in_maps


_NC_CACHE = {}


def _get_nc():
    if "nc" not in _NC_CACHE:
        _NC_CACHE["nc"] = build_nc()
    return _NC_CACHE["nc"]


def kernel(q, k, v, Wq, bq, Wk, bk, Wv, bv, Wo, bo):
    from concourse.bass_utils import run_bass_kernel_spmd

    q = np.asarray(q, dtype=np.float32)
    k = np.asarray(k, dtype=np.float32)
    v = np.asarray(v, dtype=np.float32)
    nc = _get_nc()
    in_maps = _host_inputs(q, k, v,
                           np.asarray(Wq, np.float32), np.asarray(Wk, np.float32),
                           np.asarray(Wv, np.float32), np.asarray(Wo, np.float32),
                           np.asarray(bq, np.float32), np.asarray(bk, np.float32),
                           np.asarray(bv, np.float32))
    res = run_bass_kernel_spmd(nc, in_maps, core_ids=list(range(NCORES)))
    parts = [np.asarray(r["outp"], dtype=np.float32) for r in res.results]
    out = np.stack(
        [sum(parts[b * NGROUP:(b + 1) * NGROUP]) for b in range(B)], axis=0)
    out = out + np.asarray(bo, np.float32)[None, None, :]
    return out.astype(np.float32)
